# revision 1
# baseline (speedup 1.0000x reference)
"""BiGraphSAGEDecoder Trainium2 kernel.

Sharding: 8 cores = 4 batches x {up-path, down-path}. One SPMD bass program;
the up/down asymmetry is handled purely by data (down cores receive host-
transposed adjacency / adjacency-weight matrices). Per layer, the two cores of
a batch exchange their halves of the concatenated features with a 2-rank
AllGather, then each normalizes redundantly so both hold the full h.

Math per layer (per core, its path):
  prod = adj . (Wadj . mask + unmask)            (elementwise, DVE)
  s    = prod^T @ h                              (PE, lhsT = prod blocks)
  xT   = (inv @ s)^T                             (PE, rhs = invT streamed)
  cat_own = [x @ Wcat | h @ Wbias_half]          (PE; h@Wb via PE-transposed h)
  exchange cat halves -> full cat; h' = lrelu(cat / max(||cat||, 1e-12))
Layer 2 computes only the two drug rows after s. Head: bilinear form on PE.
"""

import os
import sys
import types
import contextlib

sys.path.insert(0, "/opt/trn_rl_repo")

import numpy as np

import concourse.bass as bass
import concourse.tile as tile
from concourse import mybir, bacc
from concourse.mybir import AxisListType
from concourse.masks import make_identity
from concourse.bass_utils import run_bass_kernel_spmd

FP = mybir.dt.float32
FPR = mybir.dt.float32r
AF = mybir.ActivationFunctionType
ALU = mybir.AluOpType

# ---------------------------------------------------------------------------
# Environment patches (required for this container's toolchain)
# ---------------------------------------------------------------------------


def install_ntff_shim():
    """antenv.axon_hooks is absent in this image; provide it so trace=True
    profiling works (used by test.py, harmless otherwise)."""
    try:
        import antenv.axon_hooks  # noqa: F401
        return
    except ImportError:
        pass
    try:
        import antenv
    except ImportError:
        return
    mod = types.ModuleType("antenv.axon_hooks")
    _holder = {"hook": None}
    mod.set_axon_ntff_profile_hook = lambda h: _holder.__setitem__("hook", h)
    mod.get_axon_ntff_profile_hook = lambda: _holder["hook"]
    sys.modules["antenv.axon_hooks"] = mod
    antenv.axon_hooks = mod
    try:
        from trn_agent_boot.trn_boot import _ntff_profile_via_ctypes

        hook = _ntff_profile_via_ctypes("/opt/axon/libaxon_pjrt.so")
        if hook is not None:
            mod.set_axon_ntff_profile_hook(hook)
    except Exception:
        pass


install_ntff_shim()

if os.environ.get("KGSD_LDW_OPT", "1") != "0":
    # experiment: let walrus dedup back-to-back LDWEIGHTS
    import concourse.bass_utils as _bu
    _orig_run_command = _bu.run_command

    def _patched_run_command(argv, **kw):
        argv = ["--enable-ldw-opt=true" if a == "--enable-ldw-opt=false"
                else a for a in argv]
        return _orig_run_command(argv, **kw)

    _bu.run_command = _patched_run_command

# ---------------------------------------------------------------------------
# Problem constants
# ---------------------------------------------------------------------------

N_FULL = 2048
B = 4
P = 128
DOUT = 256     # per-path cat chunk width
BH = 128       # bias half width per core
DEC = 128
DINS = (256, 768, 768)   # per-layer input dims
EPS = 1e-12
LEAK = 0.1

JSB = 256      # mm1 column superblock (j columns per packed strip tile)
KPACK = 2      # k-tiles packed per mm1 strip tile


def _ceil_div(a, b):
    return -(-a // b)


# ---------------------------------------------------------------------------
# Program builder
# ---------------------------------------------------------------------------

class _StopBuild(Exception):
    pass


def build_program(n_cores: int, N: int = N_FULL, stop_phase: int = 99):
    """Build the SPMD bass program. Returns (nc, input_names).

    stop_phase (debug): 1=x-load only, 2=+l0 bias, 3=+l0 mm1, 4=+l0 mm2,
    5=+l0 mm3+AG+assemble, 6=+l1, 7=full.
    """
    NT = N // P                # k/j/i tiles of 128
    NPAIR = NT // KPACK        # packed k strip-pairs
    NJSB = N // JSB            # mm1 column superblocks
    MM2_JP = 512               # mm2 j' superblock width
    NJP = N // MM2_JP

    nc = bacc.Bacc("TRN2", target_bir_lowering=False, debug=False,
                   num_devices=n_cores)

    # --- DRAM I/O ---
    x_d = nc.dram_tensor("x", [N, DINS[0]], FP, kind="ExternalInput")
    adj_d = nc.dram_tensor("adj", [N, N], FP, kind="ExternalInput")
    invT_d = nc.dram_tensor("invT", [N, N], FP, kind="ExternalInput")
    wa_d = [nc.dram_tensor(f"w{l}a", [N, N], FP, kind="ExternalInput")
            for l in range(3)]
    wc_d = [nc.dram_tensor(f"w{l}c", [DINS[l], DOUT], FP, kind="ExternalInput")
            for l in range(3)]
    wb_d = [nc.dram_tensor(f"w{l}b", [DINS[l], BH], FP, kind="ExternalInput")
            for l in range(3)]
    p1_d = nc.dram_tensor("p1", [3 * DOUT, DEC], FP, kind="ExternalInput")
    p2_d = nc.dram_tensor("p2", [DEC, DEC], FP, kind="ExternalInput")
    y_d = nc.dram_tensor("ypred", [1, 1], FP, kind="ExternalOutput")

    groups = [[i, i + 1] for i in range(0, n_cores, 2)]

    try:
      with tile.TileContext(nc) as tc:
        with contextlib.ExitStack() as ctx:
            # --- pools (all opened once; tags bound memory) ---
            const_p = ctx.enter_context(tc.tile_pool(name="const", bufs=1))
            h_p = ctx.enter_context(tc.tile_pool(name="h", bufs=1))
            # s and hT share one 48KB/partition slot (disjoint lifetimes:
            # hT(l) dies before s(l) is written; s(l) dies before hT(l+1))
            s_p = ctx.enter_context(tc.tile_pool(name="s", bufs=1))
            adj_p = ctx.enter_context(tc.tile_pool(name="adjs", bufs=3))
            w_p = ctx.enter_context(tc.tile_pool(name="ws", bufs=3))
            prod_p = ctx.enter_context(tc.tile_pool(name="prod", bufs=10))
            inv_p = ctx.enter_context(tc.tile_pool(name="invs", bufs=3))
            mm3l_p = ctx.enter_context(tc.tile_pool(name="mm3l", bufs=4))
            wcb_p = ctx.enter_context(tc.tile_pool(name="wcb", bufs=2))
            misc_p = ctx.enter_context(tc.tile_pool(name="misc", bufs=3))
            norm_p = ctx.enter_context(tc.tile_pool(name="norm", bufs=3))
            psum_p = ctx.enter_context(
                tc.tile_pool(name="psum", bufs=8, space="PSUM"))
            dram_p = ctx.enter_context(
                tc.tile_pool(name="dram", bufs=2, space="DRAM"))

            ident = const_p.tile([P, P], FP, tag="ident")
            make_identity(nc, ident)

            # h as per-row-block tiles (pipelines assembly/normalize/use)
            def new_h(din):
                return [h_p.tile([P, din], FPR, tag=f"h{kt}", name="h_t")
                        for kt in range(NT)]

            h_t = new_h(DINS[0])
            # load x -> h tiles
            for kt in range(NT):
                nc.sync.dma_start(
                    h_t[kt][:],
                    x_d.ap()[kt * P:(kt + 1) * P, :].bitcast(FPR))

            drug_rows = None  # final [2,768] tile

            def _dump_and_done(src_ap):
                y_sb0 = misc_p.tile([1, 1], FP, tag="y_sb", name="y_dbg")
                nc.vector.tensor_copy(y_sb0[:], src_ap)
                nc.sync.dma_start(y_d.ap(), y_sb0[:])

            if stop_phase <= 1:
                _dump_and_done(h_t[0][0:1, 0:1])
            n_layers = 0 if stop_phase <= 1 else (
                1 if stop_phase <= 5 else (2 if stop_phase <= 6 else 3))
            for l in range(n_layers):
                din = DINS[l]
                ND = din // P
                last = (l == 2)

                # ---- weights for this layer ----
                wc_t = wcb_p.tile([P, ND * DOUT], FPR, tag="wc")
                for d in range(ND):
                    nc.scalar.dma_start(
                        wc_t[:, d * DOUT:(d + 1) * DOUT],
                        wc_d[l].ap()[d * P:(d + 1) * P, :].bitcast(FPR))
                wb_t = wcb_p.tile([P, ND * BH], FP, tag="wb")
                for d in range(ND):
                    nc.scalar.dma_start(
                        wb_t[:, d * BH:(d + 1) * BH],
                        wb_d[l].ap()[d * P:(d + 1) * P, :])

                # ---- bias chunk: hT = h^T (PE), bias = h @ Wb_half ----
                if not last:
                    # stage DRAM for own cat chunk, split in row halves so
                    # each half's AllGather can overlap the other's compute
                    stage_h = [dram_p.tile([N // 2, DOUT + BH], FP,
                                           tag=f"stage{hh}", name="stage_h")
                               for hh in range(2)]
                    hT_t = s_p.tile([P, ND * N], FP, tag="s")
                    for d in range(ND):
                        for it in range(NT):
                            pt = psum_p.tile([P, P], FP, tag="ps")
                            nc.tensor.transpose(
                                pt[:],
                                h_t[it][:, d * P:(d + 1) * P].bitcast(FP),
                                ident[:])
                            dst = hT_t[:, d * N + it * P: d * N + (it + 1) * P]
                            if it % 2 == 0:
                                nc.vector.tensor_copy(dst, pt[:])
                            else:
                                nc.scalar.copy(dst, pt[:])
                    for it in range(NT):
                        pb = psum_p.tile([P, BH], FP, tag="ps")
                        for d in range(ND):
                            nc.tensor.matmul(
                                pb[:],
                                hT_t[:, d * N + it * P: d * N + (it + 1) * P],
                                wb_t[:, d * BH:(d + 1) * BH],
                                start=(d == 0), stop=(d == ND - 1))
                        sb = misc_p.tile([P, BH], FP, tag="stg_b")
                        nc.scalar.copy(sb[:], pb[:])
                        hh, io = divmod(it, NT // 2)
                        nc.scalar.dma_start(
                            stage_h[hh][io * P:(io + 1) * P, DOUT:DOUT + BH],
                            sb[:])
                else:
                    # only rows N-2, N-1 (partition-0 copy built by l1)
                    hTd = misc_p.tile([P, ND * 2], FP, tag="hTd")
                    for d in range(ND):
                        pt = psum_p.tile([P, 2], FP, tag="ps")
                        nc.tensor.transpose(
                            pt[:],
                            hdrug_t[:, d * P:(d + 1) * P],
                            ident[0:2, 0:2])
                        nc.vector.tensor_copy(hTd[:, d * 2:(d + 1) * 2], pt[:])
                    pb = psum_p.tile([2, BH], FP, tag="ps")
                    for d in range(ND):
                        nc.tensor.matmul(
                            pb[:], hTd[:, d * 2:(d + 1) * 2],
                            wb_t[:, d * BH:(d + 1) * BH],
                            start=(d == 0), stop=(d == ND - 1))
                    stage2 = dram_p.tile([2, DOUT + BH], FP, tag="stage2")
                    sb = misc_p.tile([2, BH], FP, tag="stg_b")
                    nc.scalar.copy(sb[:], pb[:])
                    nc.scalar.dma_start(stage2[:, DOUT:DOUT + BH], sb[:])

                if stop_phase <= 2 and l == 0:
                    _dump_and_done(h_t[0][0:1, 0:1])
                    break

                # ---- mm1: s = prod^T @ h ----
                s_t = s_p.tile([P, NT * din], FPR, tag="s")
                for jsb in range(NJSB):
                    prods = []
                    for t in range(NPAIR):
                        a_t = adj_p.tile([P, KPACK, JSB], FP, tag="adj")
                        nc.sync.dma_start(
                            a_t[:],
                            adj_d.ap()[t * KPACK * P:(t + 1) * KPACK * P,
                                       jsb * JSB:(jsb + 1) * JSB]
                            .rearrange("(a p) c -> p a c", p=P))
                        ww_t = w_p.tile([P, KPACK, JSB], FP, tag="wadj")
                        nc.scalar.dma_start(
                            ww_t[:],
                            wa_d[l].ap()[t * KPACK * P:(t + 1) * KPACK * P,
                                         jsb * JSB:(jsb + 1) * JSB]
                            .rearrange("(a p) c -> p a c", p=P))
                        # mask already baked into Wadj host-side
                        pr_t = prod_p.tile([P, KPACK, JSB], FPR, tag="prod")
                        nc.vector.tensor_tensor(pr_t[:], a_t[:], ww_t[:],
                                                ALU.mult)
                        prods.append(pr_t)
                    for jl in range(JSB // P):
                        j = jsb * (JSB // P) + jl
                        pA = psum_p.tile([P, min(din, 512)], FP, tag="ps")
                        pBw = din - 512
                        pB = psum_p.tile([P, pBw], FP, tag="ps", name="pB") \
                            if pBw > 0 else None
                        for t in range(NPAIR):
                            for a in range(KPACK):
                                k = t * KPACK + a
                                lhsT = prods[t][:, a, jl * P:(jl + 1) * P]
                                st = (k == 0)
                                sp = (k == NT - 1)
                                nc.tensor.matmul(
                                    pA[:], lhsT,
                                    h_t[k][:, 0:min(din, 512)],
                                    start=st, stop=sp)
                                if pB is not None:
                                    nc.tensor.matmul(
                                        pB[:], lhsT,
                                        h_t[k][:, 512:din],
                                        start=st, stop=sp)
                        eng = nc.scalar if (j % 2 == 0) else nc.vector
                        if eng is nc.scalar:
                            nc.scalar.copy(
                                s_t[:, j * din: j * din + min(din, 512)], pA[:])
                            if pB is not None:
                                nc.scalar.copy(
                                    s_t[:, j * din + 512:(j + 1) * din], pB[:])
                        else:
                            nc.vector.tensor_copy(
                                s_t[:, j * din: j * din + min(din, 512)], pA[:])
                            if pB is not None:
                                nc.vector.tensor_copy(
                                    s_t[:, j * din + 512:(j + 1) * din], pB[:])

                if stop_phase <= 3 and l == 0:
                    _dump_and_done(s_t[0:1, 0:1])
                    break

                # ---- mm2: xT = (inv @ s)^T ----
                if not last:
                    xT_dram = dram_p.tile([din, N], FP, tag="xT")
                    for jp in range(NJP):
                        pxs = [psum_p.tile([P, MM2_JP], FP, tag="ps", name="px")
                               for _ in range(ND)]
                        for jt in range(NT):
                            r_t = inv_p.tile([P, MM2_JP], FPR, tag="inv")
                            nc.sync.dma_start(
                                r_t[:],
                                invT_d.ap()[jt * P:(jt + 1) * P,
                                            jp * MM2_JP:(jp + 1) * MM2_JP]
                                .bitcast(FPR))
                            for d in range(ND):
                                nc.tensor.matmul(
                                    pxs[d][:],
                                    s_t[:, jt * din + d * P:
                                        jt * din + (d + 1) * P],
                                    r_t[:],
                                    start=(jt == 0), stop=(jt == NT - 1))
                        for d in range(ND):
                            xs = misc_p.tile([P, MM2_JP], FPR, tag="xstg",
                                             bufs=4)
                            if d % 2 == 0:
                                nc.scalar.copy(xs[:], pxs[d][:])
                            else:
                                nc.vector.tensor_copy(xs[:], pxs[d][:])
                            nc.sync.dma_start(
                                xT_dram[d * P:(d + 1) * P,
                                        jp * MM2_JP:(jp + 1) * MM2_JP]
                                .bitcast(FPR),
                                xs[:])

                    if stop_phase <= 4 and l == 0:
                        _dump_and_done(s_t[0:1, 0:1])
                        break

                    # ---- mm3: cat_own = x @ Wc ----
                    for it in range(NT):
                        pc = psum_p.tile([P, DOUT], FP, tag="ps")
                        for d in range(ND):
                            lt = mm3l_p.tile([P, P], FPR, tag="mm3l")
                            nc.sync.dma_start(
                                lt[:],
                                xT_dram[d * P:(d + 1) * P,
                                        it * P:(it + 1) * P].bitcast(FPR))
                            nc.tensor.matmul(
                                pc[:], lt[:],
                                wc_t[:, d * DOUT:(d + 1) * DOUT],
                                start=(d == 0), stop=(d == ND - 1))
                        sc = misc_p.tile([P, DOUT], FP, tag="stg_c")
                        nc.scalar.copy(sc[:], pc[:])
                        hh, io = divmod(it, NT // 2)
                        nc.scalar.dma_start(
                            stage_h[hh][io * P:(io + 1) * P, 0:DOUT], sc[:])

                    if stop_phase <= 4.3 and l == 0:
                        _dump_and_done(s_t[0:1, 0:1])
                        break

                    # ---- exchange (two halves, overlapped) ----
                    ag_h = []
                    for hh in range(2):
                        agt = dram_p.tile([2, N // 2, DOUT + BH], FP,
                                          tag=f"ag{hh}", name="ag_h")
                        nc.gpsimd.collective_compute(
                            "AllGather", ALU.bypass, replica_groups=groups,
                            ins=[stage_h[hh].opt()], outs=[agt.opt()])
                        ag_h.append(agt)

                    if stop_phase <= 4.6 and l == 0:
                        _dump_and_done(s_t[0:1, 0:1])
                        break

                    # ---- assemble + normalize + lrelu -> new h ----
                    dnext = 3 * DOUT
                    h_t = new_h(dnext)
                    for it in range(NT):
                        hh, io = divmod(it, NT // 2)
                        ag = ag_h[hh]
                        sl = slice(io * P, (io + 1) * P)
                        ht = h_t[it]
                        nc.sync.dma_start(
                            ht[:, 0:DOUT], ag[0, sl, 0:DOUT].bitcast(FPR))
                        nc.sync.dma_start(
                            ht[:, DOUT:2 * DOUT],
                            ag[1, sl, 0:DOUT].bitcast(FPR))
                        nc.sync.dma_start(
                            ht[:, 2 * DOUT:2 * DOUT + BH],
                            ag[0, sl, DOUT:DOUT + BH].bitcast(FPR))
                        nc.sync.dma_start(
                            ht[:, 2 * DOUT + BH:3 * DOUT],
                            ag[1, sl, DOUT:DOUT + BH].bitcast(FPR))
                        ct = ht[:]
                        if stop_phase <= 4.7 and l == 0:
                            continue
                        sq = norm_p.tile([P, dnext], FPR, tag="sq")
                        ssq = norm_p.tile([P, 1], FP, tag="ssq")
                        nc.vector.tensor_tensor(sq[:], ct, ct, ALU.mult)
                        nc.vector.tensor_reduce(ssq[:], sq[:],
                                                AxisListType.X, ALU.add)
                        if stop_phase <= 4.75 and l == 0:
                            continue
                        nrm = norm_p.tile([P, 1], FP, tag="nrm")
                        nc.scalar.activation(nrm[:], ssq[:], AF.Sqrt)
                        nc.vector.tensor_scalar_max(nrm[:], nrm[:], EPS)
                        rn = norm_p.tile([P, 1], FP, tag="rn")
                        nc.vector.reciprocal(rn[:], nrm[:])
                        if stop_phase <= 4.8 and l == 0:
                            continue
                        # h = max(x, 0.1x) with x = cat/norm  (leaky relu)
                        nc.vector.tensor_scalar(ct, ct, rn[:], None, ALU.mult)
                        if stop_phase <= 4.85 and l == 0:
                            continue
                        nc.scalar.mul(sq[:], ct, LEAK)
                        nc.vector.tensor_max(ct, ct, sq[:])
                    if l == 1:
                        # extra partition-0-based copy of the two drug rows
                        # (PE ops cannot address partitions 126:128)
                        hdrug_t = misc_p.tile([2, dnext], FP, tag="hdrug", bufs=1)
                        agl = ag_h[1]
                        NH = N // 2
                        nc.sync.dma_start(hdrug_t[:, 0:DOUT],
                                          agl[0, NH - 2:NH, 0:DOUT])
                        nc.sync.dma_start(hdrug_t[:, DOUT:2 * DOUT],
                                          agl[1, NH - 2:NH, 0:DOUT])
                        nc.sync.dma_start(hdrug_t[:, 2 * DOUT:2 * DOUT + BH],
                                          agl[0, NH - 2:NH, DOUT:DOUT + BH])
                        nc.sync.dma_start(hdrug_t[:, 2 * DOUT + BH:3 * DOUT],
                                          agl[1, NH - 2:NH, DOUT:DOUT + BH])
                        dsq = norm_p.tile([2, dnext], FP, tag="sq")
                        dssq = norm_p.tile([2, 1], FP, tag="ssq")
                        nc.vector.tensor_tensor(dsq[:], hdrug_t[:],
                                                hdrug_t[:], ALU.mult)
                        nc.vector.tensor_reduce(dssq[:], dsq[:],
                                                AxisListType.X, ALU.add)
                        dnrm = norm_p.tile([2, 1], FP, tag="nrm")
                        nc.scalar.activation(dnrm[:], dssq[:], AF.Sqrt)
                        nc.vector.tensor_scalar_max(dnrm[:], dnrm[:], EPS)
                        drn = norm_p.tile([2, 1], FP, tag="rn")
                        nc.vector.reciprocal(drn[:], dnrm[:])
                        nc.vector.tensor_scalar(hdrug_t[:], hdrug_t[:],
                                                drn[:], None, ALU.mult)
                        nc.scalar.mul(dsq[:], hdrug_t[:], LEAK)
                        nc.vector.tensor_max(hdrug_t[:], hdrug_t[:], dsq[:])
                else:
                    # ---- l2: only drug rows j' in {N-2, N-1} ----
                    xT2 = misc_p.tile([P, ND * 2], FP, tag="xT2")
                    px2 = [psum_p.tile([P, 2], FP, tag="ps", name="px2")
                           for _ in range(ND)]
                    for jt in range(NT):
                        r_t = inv_p.tile([P, 2], FPR, tag="inv2")
                        nc.sync.dma_start(
                            r_t[:],
                            invT_d.ap()[jt * P:(jt + 1) * P, N - 2:N]
                            .bitcast(FPR))
                        for d in range(ND):
                            nc.tensor.matmul(
                                px2[d][:],
                                s_t[:, jt * din + d * P:jt * din + (d + 1) * P],
                                r_t[:], start=(jt == 0), stop=(jt == NT - 1))
                    for d in range(ND):
                        nc.vector.tensor_copy(xT2[:, d * 2:(d + 1) * 2],
                                              px2[d][:])
                    pc = psum_p.tile([2, DOUT], FP, tag="ps")
                    for d in range(ND):
                        nc.tensor.matmul(
                            pc[:], xT2[:, d * 2:(d + 1) * 2],
                            wc_t[:, d * DOUT:(d + 1) * DOUT].bitcast(FP),
                            start=(d == 0), stop=(d == ND - 1))
                    sc = misc_p.tile([2, DOUT], FP, tag="stg_c")
                    nc.scalar.copy(sc[:], pc[:])
                    nc.scalar.dma_start(stage2[:, 0:DOUT], sc[:])

                    ag2 = dram_p.tile([2, 2, DOUT + BH], FP, tag="ag2")
                    nc.gpsimd.collective_compute(
                        "AllGather", ALU.bypass, replica_groups=groups,
                        ins=[stage2.opt()], outs=[ag2.opt()])

                    dnext = 3 * DOUT
                    dr = norm_p.tile([2, dnext], FP, tag="drug", bufs=1)
                    nc.sync.dma_start(dr[:, 0:DOUT], ag2[0, :, 0:DOUT])
                    nc.sync.dma_start(dr[:, DOUT:2 * DOUT],
                                      ag2[1, :, 0:DOUT])
                    nc.sync.dma_start(dr[:, 2 * DOUT:2 * DOUT + BH],
                                      ag2[0, :, DOUT:DOUT + BH])
                    nc.sync.dma_start(dr[:, 2 * DOUT + BH:3 * DOUT],
                                      ag2[1, :, DOUT:DOUT + BH])
                    sq = norm_p.tile([2, dnext], FP, tag="sq")
                    ssq = norm_p.tile([2, 1], FP, tag="ssq")
                    nc.vector.tensor_tensor(sq[:], dr[:], dr[:], ALU.mult)
                    nc.vector.tensor_reduce(ssq[:], sq[:],
                                            AxisListType.X, ALU.add)
                    nrm = norm_p.tile([2, 1], FP, tag="nrm")
                    nc.scalar.activation(nrm[:], ssq[:], AF.Sqrt)
                    nc.vector.tensor_scalar_max(nrm[:], nrm[:], EPS)
                    rn = norm_p.tile([2, 1], FP, tag="rn")
                    nc.vector.reciprocal(rn[:], nrm[:])
                    nc.vector.tensor_scalar(dr[:], dr[:], rn[:], None,
                                            ALU.mult)
                    nc.scalar.mul(sq[:], dr[:], LEAK)
                    nc.vector.tensor_max(dr[:], dr[:], sq[:])
                    drug_rows = dr

            if stop_phase <= 6:
                if stop_phase >= 5 and n_layers >= 1:
                    _dump_and_done(h_t[0][0:1, 0:1])
            do_head = stop_phase > 6
            # ---- head: ypred = (a P1 P2) . (b P1) ----
            D3 = 3 * DOUT
            ND3 = D3 // P
            if do_head:
                p1_t = const_p.tile([P, ND3 * DEC], FP, tag="p1")
                for d in range(ND3):
                    nc.sync.dma_start(p1_t[:, d * DEC:(d + 1) * DEC],
                                      p1_d.ap()[d * P:(d + 1) * P, :])
                p2_t = const_p.tile([P, DEC], FP, tag="p2")
                nc.sync.dma_start(p2_t[:], p2_d.ap())
            if do_head:
                dT = misc_p.tile([P, ND3 * 2], FP, tag="dT")
                for d in range(ND3):
                    pt = psum_p.tile([P, 2], FP, tag="ps")
                    nc.tensor.transpose(pt[:], drug_rows[:, d * P:(d + 1) * P],
                                        ident[0:2, 0:2])
                    nc.vector.tensor_copy(dT[:, d * 2:(d + 1) * 2], pt[:])
                pw = psum_p.tile([P, 2], FP, tag="ps")
                for d in range(ND3):
                    nc.tensor.matmul(pw[:], p1_t[:, d * DEC:(d + 1) * DEC],
                                     dT[:, d * 2:(d + 1) * 2],
                                     start=(d == 0), stop=(d == ND3 - 1))
                w_sb = misc_p.tile([P, 2], FP, tag="w_sb")
                nc.vector.tensor_copy(w_sb[:], pw[:])
                ptt = psum_p.tile([P, 1], FP, tag="ps")
                nc.tensor.matmul(ptt[:], p2_t[:], w_sb[:, 0:1], start=True,
                                 stop=True)
                t_sb = misc_p.tile([P, 1], FP, tag="t_sb")
                nc.vector.tensor_copy(t_sb[:], ptt[:])
                py = psum_p.tile([1, 1], FP, tag="ps")
                nc.tensor.matmul(py[:], t_sb[:], w_sb[:, 1:2], start=True,
                                 stop=True)
                y_sb = misc_p.tile([1, 1], FP, tag="y_sb")
                nc.vector.tensor_copy(y_sb[:], py[:])
                nc.sync.dma_start(y_d.ap(), y_sb[:])
    except _StopBuild:
        pass

    nc.compile()
    return nc


# ---------------------------------------------------------------------------
# Host-side input prep
# ---------------------------------------------------------------------------

def make_in_maps(inputs: dict, n_cores: int):
    """Per-core input dicts. Core 2b = up path of batch b, 2b+1 = down."""
    f32c = lambda a: np.ascontiguousarray(np.asarray(a, dtype=np.float32))

    def bake_mask(w):
        w = np.array(w, dtype=np.float32)
        w[-2:, :] = 1.0
        w[:, -2:] = 1.0
        return w
    maps = []
    for c in range(n_cores):
        b, down = divmod(c, 2)
        m = {
            "x": f32c(inputs["x"][b]),
            "p1": f32c(inputs["parameter1"]),
            "p2": f32c(inputs["parameter2"]),
        }
        if not down:
            m["adj"] = f32c(inputs["adj"][b])
            m["invT"] = f32c(inputs["up_inv_deg"][b].T)
            for l in range(3):
                m[f"w{l}a"] = bake_mask(inputs[f"l{l}_up_adj_w"])
                m[f"w{l}c"] = f32c(inputs[f"l{l}_up_w"])
                m[f"w{l}b"] = f32c(inputs[f"l{l}_bias"][:, :BH])
        else:
            m["adj"] = f32c(inputs["adj"][b].T)
            m["invT"] = f32c(inputs["down_inv_deg"][b].T)
            for l in range(3):
                m[f"w{l}a"] = bake_mask(inputs[f"l{l}_down_adj_w"].T)
                m[f"w{l}c"] = f32c(inputs[f"l{l}_down_w"])
                m[f"w{l}b"] = f32c(inputs[f"l{l}_bias"][:, BH:])
        maps.append(m)
    return maps


_nc_cache = {}


def _get_program(n_cores, N):
    key = (n_cores, N)
    if key not in _nc_cache:
        _nc_cache[key] = build_program(n_cores, N)
    return _nc_cache[key]


def kernel(**inputs) -> np.ndarray:
    n_cores = 8
    nc = _get_program(n_cores, N_FULL)
    in_maps = make_in_maps(inputs, n_cores)
    res = run_bass_kernel_spmd(nc, in_maps, core_ids=list(range(n_cores)))
    out = np.zeros((B, 1), dtype=np.float32)
    for b in range(B):
        out[b, 0] = res.results[2 * b]["ypred"][0, 0]
    return out



# revision 14
# speedup vs baseline: 1.9513x; 1.9513x over previous
"""BiGraphSAGEDecoder Trainium2 kernel (v2 — restructured).

Sharding: 8 cores = 4 batches x {up-path, down-path}. One SPMD bass program;
up/down asymmetry handled by data (down cores get host-transposed matrices).

Key restructurings vs v1:
  - mm1 emits s^T directly (lhsT = h blocks, rhs = prod strips), enabling
    the reassociation x' = inv @ (s @ Wc): the big [N,N] matmul contracts
    against 256 cols instead of din=768.
  - layer 2 never computes s: x2 = ivr @ prod^T @ h is folded as
    u = ivr @ prodT (prodT streamed from host-transposed adj/wadj),
    w = u^T, x2 = w^T @ h.
  - full bias chunk computed locally (no bias in the exchange); AllGather
    moves only the 256-col own chunk per rank, in fp16.
  - dtypes: adj/wadj/inv fp16 in DRAM, prod fp16, h fp16, y fp16,
    sT fp32r, weights fp32r, psum fp32. Validated max rel err 1.9e-3
    vs fp32 reference (tolerance 2e-2).

Math per layer (per core, its path):
  prod = adj .* wadj_baked                  (DVE, fp16)
  sT   = (prod^T @ h)^T  via lhsT=h         (PE)
  y    = s @ Wc                             (PE, lhsT = sT blocks)
  own  = inv @ y                            (PE, lhsT = invT blocks)
  bias = h @ Wb (full)                      (PE, via PE-transposed h)
  exchange own chunks (2-rank AllGather, two halves, fp16)
  h' = lrelu(cat / max(||cat||, eps))       (DVE/scalar/gpsimd)
"""

import os
import sys
import types
import contextlib

sys.path.insert(0, "/opt/trn_rl_repo")

import numpy as np

import concourse.bass as bass
import concourse.tile as tile
from concourse import mybir, bacc
from concourse.mybir import AxisListType
from concourse.masks import make_identity
from concourse.bass_utils import run_bass_kernel_spmd

FP = mybir.dt.float32
FPR = mybir.dt.float32r
F16 = mybir.dt.float16
AF = mybir.ActivationFunctionType
ALU = mybir.AluOpType

# ---------------------------------------------------------------------------
# Environment patches (required for this container's toolchain)
# ---------------------------------------------------------------------------


def install_ntff_shim():
    """antenv.axon_hooks is absent in this image; provide it so trace=True
    profiling works (used by test.py, harmless otherwise)."""
    try:
        import antenv.axon_hooks  # noqa: F401
        return
    except ImportError:
        pass
    try:
        import antenv
    except ImportError:
        return
    mod = types.ModuleType("antenv.axon_hooks")
    _holder = {"hook": None}
    mod.set_axon_ntff_profile_hook = lambda h: _holder.__setitem__("hook", h)
    mod.get_axon_ntff_profile_hook = lambda: _holder["hook"]
    sys.modules["antenv.axon_hooks"] = mod
    antenv.axon_hooks = mod
    try:
        from trn_agent_boot.trn_boot import _ntff_profile_via_ctypes

        hook = _ntff_profile_via_ctypes("/opt/axon/libaxon_pjrt.so")
        if hook is not None:
            mod.set_axon_ntff_profile_hook(hook)
    except Exception:
        pass


install_ntff_shim()

if os.environ.get("KGSD_LDW_OPT", "0") != "0":
    # let walrus dedup back-to-back LDWEIGHTS
    import concourse.bass_utils as _bu
    _orig_run_command = _bu.run_command

    def _patched_run_command(argv, **kw):
        argv = ["--enable-ldw-opt=true" if a == "--enable-ldw-opt=false"
                else a for a in argv]
        return _orig_run_command(argv, **kw)

    _bu.run_command = _patched_run_command

# ---------------------------------------------------------------------------
# Problem constants
# ---------------------------------------------------------------------------

N_FULL = 2048
B = 4
P = 128
DOUT = 256     # per-path cat chunk width (also bias width)
DEC = 128
DINS = (256, 768, 768)   # per-layer input dims
D3 = 768
EPS = 1e-12
LEAK = 0.1
JB = 512       # mm1 / x' column block (one PSUM bank of fp32)


# ---------------------------------------------------------------------------
# Program builder
# ---------------------------------------------------------------------------


def build_program(n_cores: int, N: int = N_FULL, stop_phase: int = 99):
    """Build the SPMD bass program.

    stop_phase (debug): 1=x load, 2=l0 mm1, 3=l0 y, 4=l0 x', 5=l0 done,
    6=l1 done, 7=full.
    """
    NT = N // P              # 128-row tiles
    NJB = N // JB            # mm1/x' column blocks
    HALF = NT // 2
    NQ = max(N // JB, 1)     # aw quarter count (512-wide pairs)

    nc = bacc.Bacc("TRN2", target_bir_lowering=False, debug=False,
                   num_devices=n_cores)

    # --- DRAM I/O ---
    x_d = nc.dram_tensor("x", [N, DINS[0]], F16, kind="ExternalInput")
    # aw{l}: interleaved quarters [adj[:,q*512:(q+1)*512] | wa[:,q*512:...]]
    aw_d = [nc.dram_tensor(f"aw{l}", [N, 2 * N], F16, kind="ExternalInput")
            for l in range(3)]
    invT_d = nc.dram_tensor("invT", [N, N], F16, kind="ExternalInput")
    ivrT_d = nc.dram_tensor("ivrT", [N, 2], F16, kind="ExternalInput")
    wc_d = [nc.dram_tensor(f"w{l}c", [DINS[l], DOUT], FP, kind="ExternalInput")
            for l in range(3)]
    wb_d = [nc.dram_tensor(f"w{l}b", [DINS[l], DOUT], FP, kind="ExternalInput")
            for l in range(3)]
    p1_d = nc.dram_tensor("p1", [D3, DEC], FP, kind="ExternalInput")
    p2_d = nc.dram_tensor("p2", [DEC, DEC], FP, kind="ExternalInput")
    y_d = nc.dram_tensor("ypred", [1, 1], FP, kind="ExternalOutput")
    dbg_d = (nc.dram_tensor("dbg", [N, D3], FP, kind="ExternalOutput")
             if stop_phase < 99 else None)

    groups = [[i, i + 1] for i in range(0, n_cores, 2)]

    with tile.TileContext(nc) as tc:
        with contextlib.ExitStack() as ctx:
            const_p = ctx.enter_context(tc.tile_pool(name="const", bufs=1))
            h_p = ctx.enter_context(tc.tile_pool(name="h", bufs=1))
            sT_p = ctx.enter_context(tc.tile_pool(name="sT", bufs=1))
            y_p = ctx.enter_context(tc.tile_pool(name="y", bufs=1))
            aw_p = ctx.enter_context(tc.tile_pool(name="aw", bufs=6))
            prod_p = ctx.enter_context(tc.tile_pool(name="prod", bufs=6))
            inv_p = ctx.enter_context(tc.tile_pool(name="invs", bufs=3))
            hT_p = ctx.enter_context(tc.tile_pool(name="hT", bufs=3))
            stg_p = ctx.enter_context(tc.tile_pool(name="stg", bufs=4))
            norm_p = ctx.enter_context(tc.tile_pool(name="norm", bufs=2))
            misc_p = ctx.enter_context(tc.tile_pool(name="misc", bufs=1))
            psum_p = ctx.enter_context(
                tc.tile_pool(name="psum", bufs=8, space="PSUM"))
            dram_p = ctx.enter_context(
                tc.tile_pool(name="dram", bufs=2, space="DRAM"))

            ident = const_p.tile([P, P], FP, tag="ident")
            make_identity(nc, ident)
            ident16 = const_p.tile([P, P], F16, tag="ident16")
            nc.scalar.copy(ident16[:], ident[:])

            # --- constant loads (scalar queue) ---
            wc_t, wb_t = [], []
            for l in range(3):
                ND = DINS[l] // P
                wct = const_p.tile([P, ND * DOUT], FPR, tag=f"wc{l}")
                wbt = const_p.tile([P, ND * DOUT], FPR, tag=f"wb{l}")
                for d in range(ND):
                    nc.scalar.dma_start(
                        wct[:, d * DOUT:(d + 1) * DOUT],
                        wc_d[l].ap()[d * P:(d + 1) * P, :].bitcast(FPR))
                    nc.scalar.dma_start(
                        wbt[:, d * DOUT:(d + 1) * DOUT],
                        wb_d[l].ap()[d * P:(d + 1) * P, :].bitcast(FPR))
                wc_t.append(wct)
                wb_t.append(wbt)
            p1_t = const_p.tile([P, (D3 // P) * DEC], FP, tag="p1")
            for d in range(D3 // P):
                nc.scalar.dma_start(p1_t[:, d * DEC:(d + 1) * DEC],
                                    p1_d.ap()[d * P:(d + 1) * P, :])
            p2_t = const_p.tile([P, DEC], FP, tag="p2")
            nc.scalar.dma_start(p2_t[:], p2_d.ap())
            ivrT_t = const_p.tile([P, NT, 2], F16, tag="ivrT")
            nc.scalar.dma_start(
                ivrT_t[:],
                ivrT_d.ap().rearrange("(a p) c -> p a c", p=P))

            # --- h double generation (fp16) ---
            h_e = [h_p.tile([P, D3], F16, tag=f"he{k}", name="h_e")
                   for k in range(NT)]
            h_o = [h_p.tile([P, D3], F16, tag=f"ho{k}", name="h_o")
                   for k in range(NT)]
            for kt in range(NT):
                nc.sync.dma_start(h_e[kt][:, 0:DINS[0]],
                                  x_d.ap()[kt * P:(kt + 1) * P, :])

            sT = [sT_p.tile([P, N], FPR, tag=f"sT{d}", name="sT")
                  for d in range(6)]
            y_t = [y_p.tile([P, DOUT], F16, tag=f"y{j}", name="y_t")
                   for j in range(NT)]
            w_sb = misc_p.tile([P, 2 * NT], F16, tag="w_sb")

            def _dump_and_done(src_ap):
                y_sb0 = misc_p.tile([1, 1], FP, tag="y_dbg")
                nc.vector.tensor_copy(y_sb0[:], src_ap)
                nc.sync.dma_start(y_d.ap(), y_sb0[:])

            # l2 prefold bursts: split quarters between the two boundaries
            burst_q = [list(range(NQ))[:(NQ + 1) // 2],
                       list(range(NQ))[(NQ + 1) // 2:]]

            def l2_burst(qlist, hcur_unused):
                for qg in qlist:
                    pu = psum_p.tile([P, JB], FP, tag="ps", name="pu")
                    for j in range(NT):
                        aw2t = aw_p.tile([P, 2 * JB], F16, tag="aw")
                        nc.sync.dma_start(
                            aw2t[:],
                            aw_d[2].ap()[j * P:(j + 1) * P,
                                         qg * 2 * JB:(qg + 1) * 2 * JB])
                        pr = prod_p.tile([P, JB], F16, tag="prod")
                        nc.vector.tensor_tensor(pr[:], aw2t[:, 0:JB],
                                                aw2t[:, JB:2 * JB], ALU.mult)
                        nc.tensor.matmul(pu[0:2, :], ivrT_t[:, j, :],
                                         pr[:], start=(j == 0),
                                         stop=(j == NT - 1))
                    usb = misc_p.tile([2, JB], FPR, tag="usb")
                    nc.scalar.copy(usb[:], pu[0:2, :])
                    for kk in range(JB // P):
                        k = qg * (JB // P) + kk
                        if k >= NT:
                            break
                        ptw = psum_p.tile([P, 2], FP, tag="ps", name="ptw")
                        nc.tensor.transpose(
                            ptw[:], usb[:, kk * P:(kk + 1) * P].bitcast(FP),
                            ident[0:2, 0:2])
                        nc.vector.tensor_copy(w_sb[:, 2 * k:2 * k + 2], ptw[:])

            if stop_phase <= 1:
                _dump_and_done(h_e[0][0:1, 0:1])

            hcur, hnxt = h_e, h_o
            n_layers = 0 if stop_phase <= 1 else (2 if stop_phase <= 6 else 2)
            for l in range(2):
                if stop_phase <= 1:
                    break
                din = DINS[l]
                ND = din // P

                # ---- mm1: sT = (prod^T @ h)^T ----
                for jb in range(NJB):
                    pss = [psum_p.tile([P, JB], FP, tag="ps", name="pmm1")
                           for _ in range(ND)]
                    for k in range(NT):
                        awt = aw_p.tile([P, 2 * JB], F16, tag="aw")
                        nc.sync.dma_start(
                            awt[:],
                            aw_d[l].ap()[k * P:(k + 1) * P,
                                         jb * 2 * JB:(jb + 1) * 2 * JB])
                        pr = prod_p.tile([P, JB], F16, tag="prod")
                        nc.vector.tensor_tensor(pr[:], awt[:, 0:JB],
                                                awt[:, JB:2 * JB], ALU.mult)
                        for d in range(ND):
                            nc.tensor.matmul(
                                pss[d][:], hcur[k][:, d * P:(d + 1) * P],
                                pr[:], start=(k == 0), stop=(k == NT - 1))
                    for d in range(ND):
                        dst = sT[d][:, jb * JB:(jb + 1) * JB]
                        if d % 2 == 0:
                            nc.scalar.copy(dst, pss[d][:])
                        else:
                            nc.vector.tensor_copy(dst, pss[d][:])

                if stop_phase == 2 and l == 0:
                    _dump_and_done(sT[0][0:1, 0:1])
                    break

                # ---- y = s @ Wc  (fp16) ----
                for j in range(NT):
                    py = psum_p.tile([P, DOUT], FP, tag="ps", name="py")
                    for d in range(ND):
                        nc.tensor.matmul(
                            py[:], sT[d][:, j * P:(j + 1) * P],
                            wc_t[l][:, d * DOUT:(d + 1) * DOUT],
                            start=(d == 0), stop=(d == ND - 1))
                    nc.scalar.copy(y_t[j][:], py[:])

                if stop_phase == 3 and l == 0:
                    _dump_and_done(y_t[0][0:1, 0:1])
                    break

                # ---- x' = inv @ y : own cat chunk ----
                pxs = [psum_p.tile([P, JB], FP, tag="ps", name="px")
                       for _ in range((NT + 1) // 2)]
                for px in pxs:
                    nc.vector.memset(px[:], 0.0)
                for j in range(NT):
                    ivt = inv_p.tile([P, N], F16, tag="inv")
                    nc.sync.dma_start(ivt[:],
                                      invT_d.ap()[j * P:(j + 1) * P, :])
                    for i in range(NT):
                        px = pxs[i // 2]
                        sl = slice((i % 2) * DOUT, (i % 2 + 1) * DOUT)
                        nc.tensor.matmul(
                            px[:, sl], ivt[:, i * P:(i + 1) * P], y_t[j][:],
                            start=False, stop=(j == NT - 1),
                            skip_group_check=True)

                if stop_phase == 4 and l == 0:
                    _dump_and_done(y_t[0][0:1, 0:1])
                    break

                # ---- stage + AllGather (two halves) ----
                stage_d = [dram_p.tile([N // 2, DOUT], F16, tag=f"stg{hh}",
                                       name="stage_d") for hh in range(2)]
                for i in range(NT):
                    st = stg_p.tile([P, DOUT], F16, tag="stg")
                    nc.scalar.copy(st[:],
                                   pxs[i // 2][:, (i % 2) * DOUT:
                                               (i % 2 + 1) * DOUT])
                    hh, io = divmod(i, HALF)
                    nc.scalar.dma_start(
                        stage_d[hh][io * P:(io + 1) * P, :], st[:])
                ag_t = []
                for hh in range(2):
                    agt = dram_p.tile([2, N // 2, DOUT], F16, tag=f"ag{hh}",
                                      name="ag_t")
                    nc.gpsimd.collective_compute(
                        "AllGather", ALU.bypass, replica_groups=groups,
                        ins=[stage_d[hh].opt()], outs=[agt.opt()])
                    ag_t.append(agt)

                # ---- bias = h @ Wb (full, local) -> h' cols 512:768 ----
                for it in range(NT):
                    pb = psum_p.tile([P, DOUT], FP, tag="ps", name="pb")
                    for d in range(ND):
                        ptr = psum_p.tile([P, P], F16, tag="ps", name="ptr")
                        nc.tensor.transpose(
                            ptr[:], hcur[it][:, d * P:(d + 1) * P],
                            ident16[:])
                        hTt = hT_p.tile([P, P], FPR, tag="hT")
                        nc.vector.tensor_copy(hTt[:], ptr[:])
                        nc.tensor.matmul(
                            pb[:], hTt[:],
                            wb_t[l][:, d * DOUT:(d + 1) * DOUT],
                            start=(d == 0), stop=(d == ND - 1))
                    nc.scalar.copy(hnxt[it][:, 2 * DOUT:3 * DOUT], pb[:])

                # ---- l2 prefold burst (fills AG latency) ----
                l2_burst(burst_q[l], hcur)

                # ---- assemble + normalize + lrelu -> h' ----
                for it in range(NT):
                    hh, io = divmod(it, HALF)
                    ag = ag_t[hh]
                    ht = hnxt[it]
                    nc.sync.dma_start(ht[:, 0:DOUT],
                                      ag[0, io * P:(io + 1) * P, :])
                    nc.sync.dma_start(ht[:, DOUT:2 * DOUT],
                                      ag[1, io * P:(io + 1) * P, :])
                    if stop_phase == 4.75 and l == 0:
                        dbf = misc_p.tile([P, D3], FP, tag="dbf", bufs=2)
                        nc.vector.tensor_copy(dbf[:], ht[:])
                        nc.sync.dma_start(
                            dbg_d.ap()[it * P:(it + 1) * P, :], dbf[:])
                        continue
                    sq = norm_p.tile([P, D3], FP, tag="sq")
                    nc.gpsimd.tensor_tensor(sq[:], ht[:], ht[:], ALU.mult)
                    ssq = norm_p.tile([P, 1], FP, tag="ssq")
                    nc.vector.tensor_reduce(ssq[:], sq[:],
                                            AxisListType.X, ALU.add)
                    nrm = norm_p.tile([P, 1], FP, tag="nrm")
                    nc.scalar.activation(nrm[:], ssq[:], AF.Sqrt)
                    nc.vector.tensor_scalar_max(nrm[:], nrm[:], EPS)
                    rn = norm_p.tile([P, 1], FP, tag="rn")
                    nc.vector.reciprocal(rn[:], nrm[:])
                    if stop_phase == 4.8 and l == 0 and it == 0:
                        _dump_and_done(rn[0:1, 0:1])
                    nc.vector.tensor_scalar(ht[:], ht[:], rn[:], None,
                                            ALU.mult)
                    if stop_phase == 4.9 and l == 0 and it == 0:
                        _dump_and_done(ht[0:1, 0:1])
                    lk = norm_p.tile([P, D3], F16, tag="lk")
                    nc.scalar.mul(lk[:], ht[:], LEAK)
                    nc.vector.tensor_max(ht[:], ht[:], lk[:])

                if stop_phase == 4.75 and l == 0:
                    break
                hcur, hnxt = hnxt, hcur
                if stop_phase == 5 and l == 0:
                    _dump_and_done(hcur[0][0:1, 0:1])
                    for it in range(NT):
                        dbf = misc_p.tile([P, D3], FP, tag="dbf", bufs=2)
                        nc.vector.tensor_copy(dbf[:], hcur[it][:])
                        nc.sync.dma_start(
                            dbg_d.ap()[it * P:(it + 1) * P, :], dbf[:])
                    break

            do_tail = stop_phase > 6
            if do_tail:
                # ---- l2: x2 = w^T @ h ----
                psA = psum_p.tile([P, JB], FP, tag="ps", name="psA")
                psB = psum_p.tile([P, JB], FP, tag="ps", name="psB")
                for k in range(NT):
                    nc.tensor.matmul(psA[0:2, :], w_sb[:, 2 * k:2 * k + 2],
                                     hcur[k][:, 0:JB], start=(k == 0),
                                     stop=(k == NT - 1))
                    nc.tensor.matmul(psB[0:2, 0:D3 - JB],
                                     w_sb[:, 2 * k:2 * k + 2],
                                     hcur[k][:, JB:D3], start=(k == 0),
                                     stop=(k == NT - 1))
                x2sb = misc_p.tile([2, D3], FPR, tag="x2sb")
                nc.scalar.copy(x2sb[:, 0:JB], psA[0:2, :])
                nc.scalar.copy(x2sb[:, JB:D3], psB[0:2, 0:D3 - JB])

                # own2 = x2 @ Wc2
                pc2 = psum_p.tile([P, DOUT], FP, tag="ps", name="pc2")
                for d in range(D3 // P):
                    ptx = psum_p.tile([P, 2], FP, tag="ps", name="ptx")
                    nc.tensor.transpose(
                        ptx[:], x2sb[:, d * P:(d + 1) * P].bitcast(FP),
                        ident[0:2, 0:2])
                    x2T = hT_p.tile([P, 2], FPR, tag="x2T")
                    nc.vector.tensor_copy(x2T[:], ptx[:])
                    nc.tensor.matmul(pc2[0:2, :], x2T[:],
                                     wc_t[2][:, d * DOUT:(d + 1) * DOUT],
                                     start=(d == 0), stop=(d == D3 // P - 1))

                # bias2 = hdrug @ Wb2 (hdrug via DRAM bounce, post-norm rows)
                bounce = dram_p.tile([2, D3], F16, tag="bounce")
                nc.sync.dma_start(bounce[:], hcur[NT - 1][P - 2:P, :])
                hdr = misc_p.tile([2, D3], F16, tag="hdrug")
                nc.sync.dma_start(hdr[:], bounce[:])
                pb2 = psum_p.tile([P, DOUT], FP, tag="ps", name="pb2")
                for d in range(D3 // P):
                    ptr = psum_p.tile([P, 2], F16, tag="ps", name="ptr2")
                    nc.tensor.transpose(ptr[:], hdr[:, d * P:(d + 1) * P],
                                        ident16[0:2, 0:2])
                    hTt = hT_p.tile([P, 2], FPR, tag="x2T")
                    nc.vector.tensor_copy(hTt[:], ptr[:])
                    nc.tensor.matmul(pb2[0:2, :], hTt[:],
                                     wb_t[2][:, d * DOUT:(d + 1) * DOUT],
                                     start=(d == 0), stop=(d == D3 // P - 1))

                # stage2 + AG + assemble + norm
                stg2 = stg_p.tile([2, DOUT], F16, tag="stg")
                nc.scalar.copy(stg2[:], pc2[0:2, :])
                st2d = dram_p.tile([2, DOUT], F16, tag="stg2")
                nc.scalar.dma_start(st2d[:], stg2[:])
                ag2 = dram_p.tile([2, 2, DOUT], F16, tag="ag2")
                nc.gpsimd.collective_compute(
                    "AllGather", ALU.bypass, replica_groups=groups,
                    ins=[st2d.opt()], outs=[ag2.opt()])
                asm2 = stg_p.tile([2, 2 * DOUT], F16, tag="asm2")
                nc.sync.dma_start(asm2[:, 0:DOUT], ag2[0, :, :])
                nc.sync.dma_start(asm2[:, DOUT:2 * DOUT], ag2[1, :, :])
                dr = misc_p.tile([2, D3], FP, tag="drug")
                nc.vector.tensor_copy(dr[:, 0:2 * DOUT], asm2[:])
                nc.vector.tensor_copy(dr[:, 2 * DOUT:D3], pb2[0:2, :])
                sq = norm_p.tile([2, D3], FP, tag="sq2")
                ssq = norm_p.tile([2, 1], FP, tag="ssq2")
                nc.vector.tensor_tensor(sq[:], dr[:], dr[:], ALU.mult)
                nc.vector.tensor_reduce(ssq[:], sq[:], AxisListType.X,
                                        ALU.add)
                nrm = norm_p.tile([2, 1], FP, tag="nrm2")
                nc.scalar.activation(nrm[:], ssq[:], AF.Sqrt)
                nc.vector.tensor_scalar_max(nrm[:], nrm[:], EPS)
                rn = norm_p.tile([2, 1], FP, tag="rn2")
                nc.vector.reciprocal(rn[:], nrm[:])
                nc.vector.tensor_scalar(dr[:], dr[:], rn[:], None, ALU.mult)
                nc.scalar.mul(sq[:], dr[:], LEAK)
                nc.vector.tensor_max(dr[:], dr[:], sq[:])

                # ---- head: ypred = (a P1 P2) . (b P1) ----
                ND3 = D3 // P
                dT = misc_p.tile([P, ND3 * 2], FP, tag="dT")
                for d in range(ND3):
                    pt = psum_p.tile([P, 2], FP, tag="ps", name="phd")
                    nc.tensor.transpose(pt[:], dr[:, d * P:(d + 1) * P],
                                        ident[0:2, 0:2])
                    nc.vector.tensor_copy(dT[:, d * 2:(d + 1) * 2], pt[:])
                pw = psum_p.tile([P, 2], FP, tag="ps", name="pw")
                for d in range(ND3):
                    nc.tensor.matmul(pw[:], p1_t[:, d * DEC:(d + 1) * DEC],
                                     dT[:, d * 2:(d + 1) * 2],
                                     start=(d == 0), stop=(d == ND3 - 1))
                w_hd = misc_p.tile([P, 2], FP, tag="w_hd")
                nc.vector.tensor_copy(w_hd[:], pw[:])
                ptt = psum_p.tile([P, 1], FP, tag="ps", name="ptt")
                nc.tensor.matmul(ptt[:], p2_t[:], w_hd[:, 0:1], start=True,
                                 stop=True)
                t_sb = misc_p.tile([P, 1], FP, tag="t_sb")
                nc.vector.tensor_copy(t_sb[:], ptt[:])
                py_ = psum_p.tile([1, 1], FP, tag="ps", name="pyf")
                nc.tensor.matmul(py_[:], t_sb[:], w_hd[:, 1:2], start=True,
                                 stop=True)
                y_sb = misc_p.tile([1, 1], FP, tag="y_sb")
                nc.vector.tensor_copy(y_sb[:], py_[:])
                nc.sync.dma_start(y_d.ap(), y_sb[:])
            elif stop_phase > 5:
                _dump_and_done(hcur[0][0:1, 0:1])

    nc.compile()
    return nc


# ---------------------------------------------------------------------------
# Host-side input prep
# ---------------------------------------------------------------------------

def _pack_quarters(a16, w16, N):
    """Interleave 512-col quarters: [a[:,q] | w[:,q]] -> [N, 2N] fp16."""
    NQ = max(N // JB, 1)
    q = min(JB, N)
    out = np.empty((N, 2 * N), dtype=np.float16)
    for i in range(NQ):
        out[:, i * 2 * q:i * 2 * q + q] = a16[:, i * q:(i + 1) * q]
        out[:, i * 2 * q + q:(i + 1) * 2 * q] = w16[:, i * q:(i + 1) * q]
    return out


def make_in_maps(inputs: dict, n_cores: int, N: int = None):
    """Per-core input dicts. Core 2b = up path of batch b, 2b+1 = down."""
    if N is None:
        N = np.asarray(inputs["adj"]).shape[-1]
    f32c = lambda a: np.ascontiguousarray(np.asarray(a, dtype=np.float32))
    f16c = lambda a: np.ascontiguousarray(np.asarray(a).astype(np.float16))

    def bake_mask(w):
        w = np.array(w, dtype=np.float32)
        w[-2:, :] = 1.0
        w[:, -2:] = 1.0
        return w

    maps = []
    for c in range(n_cores):
        b, down = divmod(c, 2)
        if not down:
            A = np.asarray(inputs["adj"][b])
            IV = np.asarray(inputs["up_inv_deg"][b])
            was = [bake_mask(inputs[f"l{l}_up_adj_w"]) for l in range(3)]
            wcs = [inputs[f"l{l}_up_w"] for l in range(3)]
        else:
            A = np.asarray(inputs["adj"][b]).T
            IV = np.asarray(inputs["down_inv_deg"][b])
            was = [bake_mask(inputs[f"l{l}_down_adj_w"]).T for l in range(3)]
            wcs = [inputs[f"l{l}_down_w"] for l in range(3)]
        A16 = f16c(A)
        m = {
            "x": f16c(inputs["x"][b]),
            "invT": f16c(IV.T),
            "ivrT": f16c(IV[-2:, :].T),
            "p1": f32c(inputs["parameter1"]),
            "p2": f32c(inputs["parameter2"]),
        }
        for l in range(2):
            m[f"aw{l}"] = _pack_quarters(A16, f16c(was[l]), N)
        # l2: prodT = adjT .* waT (transpose of this core's own matrices)
        m["aw2"] = _pack_quarters(
            np.ascontiguousarray(A16.T), f16c(was[2].T), N)
        for l in range(3):
            m[f"w{l}c"] = f32c(wcs[l])
            m[f"w{l}b"] = f32c(inputs[f"l{l}_bias"])
        maps.append(m)
    return maps


_nc_cache = {}


def _get_program(n_cores, N):
    key = (n_cores, N)
    if key not in _nc_cache:
        _nc_cache[key] = build_program(n_cores, N)
    return _nc_cache[key]


def kernel(**inputs) -> np.ndarray:
    n_cores = 8
    nc = _get_program(n_cores, N_FULL)
    in_maps = make_in_maps(inputs, n_cores)
    res = run_bass_kernel_spmd(nc, in_maps, core_ids=list(range(n_cores)))
    out = np.zeros((B, 1), dtype=np.float32)
    for b in range(B):
        out[b, 0] = res.results[2 * b]["ypred"][0, 0]
    return out


# revision 24
# speedup vs baseline: 2.1612x; 1.1076x over previous
"""BiGraphSAGEDecoder Trainium2 kernel (v2 — restructured).

Sharding: 8 cores = 4 batches x {up-path, down-path}. One SPMD bass program;
up/down asymmetry handled by data (down cores get host-transposed matrices).

Key restructurings vs v1:
  - mm1 emits s^T directly (lhsT = h blocks, rhs = prod strips), enabling
    the reassociation x' = inv @ (s @ Wc): the big [N,N] matmul contracts
    against 256 cols instead of din=768.
  - layer 2 never computes s: x2 = ivr @ prod^T @ h is folded as
    u = ivr @ prodT (prodT streamed from host-transposed adj/wadj),
    w = u^T, x2 = w^T @ h.
  - full bias chunk computed locally (no bias in the exchange); AllGather
    moves only the 256-col own chunk per rank, in fp16.
  - dtypes: adj/wadj/inv fp16 in DRAM, prod fp16, h fp16, y fp16,
    sT fp32r, weights fp32r, psum fp32. Validated max rel err 1.9e-3
    vs fp32 reference (tolerance 2e-2).

Math per layer (per core, its path):
  prod = adj .* wadj_baked                  (DVE, fp16)
  sT   = (prod^T @ h)^T  via lhsT=h         (PE)
  y    = s @ Wc                             (PE, lhsT = sT blocks)
  own  = inv @ y                            (PE, lhsT = invT blocks)
  bias = h @ Wb (full)                      (PE, via PE-transposed h)
  exchange own chunks (2-rank AllGather, two halves, fp16)
  h' = lrelu(cat / max(||cat||, eps))       (DVE/scalar/gpsimd)
"""

import os
import sys
import types
import contextlib

sys.path.insert(0, "/opt/trn_rl_repo")

import numpy as np

import concourse.bass as bass
import concourse.tile as tile
from concourse import mybir, bacc
from concourse.mybir import AxisListType
from concourse.masks import make_identity
from concourse.bass_utils import run_bass_kernel_spmd

FP = mybir.dt.float32
FPR = mybir.dt.float32r
F16 = mybir.dt.float16
AF = mybir.ActivationFunctionType
ALU = mybir.AluOpType

# ---------------------------------------------------------------------------
# Environment patches (required for this container's toolchain)
# ---------------------------------------------------------------------------


def install_ntff_shim():
    """antenv.axon_hooks is absent in this image; provide it so trace=True
    profiling works (used by test.py, harmless otherwise)."""
    try:
        import antenv.axon_hooks  # noqa: F401
        return
    except ImportError:
        pass
    try:
        import antenv
    except ImportError:
        return
    mod = types.ModuleType("antenv.axon_hooks")
    _holder = {"hook": None}
    mod.set_axon_ntff_profile_hook = lambda h: _holder.__setitem__("hook", h)
    mod.get_axon_ntff_profile_hook = lambda: _holder["hook"]
    sys.modules["antenv.axon_hooks"] = mod
    antenv.axon_hooks = mod
    try:
        from trn_agent_boot.trn_boot import _ntff_profile_via_ctypes

        hook = _ntff_profile_via_ctypes("/opt/axon/libaxon_pjrt.so")
        if hook is not None:
            mod.set_axon_ntff_profile_hook(hook)
    except Exception:
        pass


install_ntff_shim()

if os.environ.get("KGSD_LDW_OPT", "0") != "0":
    # let walrus dedup back-to-back LDWEIGHTS
    import concourse.bass_utils as _bu
    _orig_run_command = _bu.run_command

    def _patched_run_command(argv, **kw):
        argv = ["--enable-ldw-opt=true" if a == "--enable-ldw-opt=false"
                else a for a in argv]
        return _orig_run_command(argv, **kw)

    _bu.run_command = _patched_run_command

# ---------------------------------------------------------------------------
# Problem constants
# ---------------------------------------------------------------------------

N_FULL = 2048
B = 4
P = 128
DOUT = 256     # per-path cat chunk width (also bias width)
DEC = 128
DINS = (256, 768, 768)   # per-layer input dims
D3 = 768
EPS = 1e-12
LEAK = 0.1
JB = 512       # mm1 / x' column block (one PSUM bank of fp32)


# ---------------------------------------------------------------------------
# Program builder
# ---------------------------------------------------------------------------


def build_program(n_cores: int, N: int = N_FULL, stop_phase: int = 99):
    """Build the SPMD bass program.

    stop_phase (debug): 1=x load, 2=l0 mm1, 3=l0 y, 4=l0 x', 5=l0 done,
    6=l1 done, 7=full.
    """
    NT = N // P              # 128-row tiles
    NJB = N // JB            # mm1/x' column blocks
    HALF = NT // 2
    NQ = max(N // JB, 1)     # aw quarter count (512-wide pairs)

    nc = bacc.Bacc("TRN2", target_bir_lowering=False, debug=False,
                   num_devices=n_cores)

    # --- DRAM I/O ---
    x_d = nc.dram_tensor("x", [N, DINS[0]], F16, kind="ExternalInput")
    # aw{l}: interleaved quarters [adj[:,q*512:(q+1)*512] | wa[:,q*512:...]]
    aw_d = [nc.dram_tensor(f"aw{l}", [N, 2 * N], F16, kind="ExternalInput")
            for l in range(3)]
    invT_d = nc.dram_tensor("invT", [N, N], F16, kind="ExternalInput")
    ivrT_d = nc.dram_tensor("ivrT", [N, 2], F16, kind="ExternalInput")
    wc_d = [nc.dram_tensor(f"w{l}c", [DINS[l], DOUT], FP, kind="ExternalInput")
            for l in range(3)]
    wb_d = [nc.dram_tensor(f"w{l}b", [DINS[l], DOUT], FP, kind="ExternalInput")
            for l in range(3)]
    p1_d = nc.dram_tensor("p1", [D3, DEC], FP, kind="ExternalInput")
    p2_d = nc.dram_tensor("p2", [DEC, DEC], FP, kind="ExternalInput")
    y_d = nc.dram_tensor("ypred", [1, 1], FP, kind="ExternalOutput")
    dbg_d = (nc.dram_tensor("dbg", [N, D3], FP, kind="ExternalOutput")
             if stop_phase < 99 else None)

    groups = [[i, i + 1] for i in range(0, n_cores, 2)]

    with tile.TileContext(nc) as tc:
        with contextlib.ExitStack() as ctx:
            const_p = ctx.enter_context(tc.tile_pool(name="const", bufs=1))
            h_p = ctx.enter_context(tc.tile_pool(name="h", bufs=1))
            sT_p = ctx.enter_context(tc.tile_pool(name="sT", bufs=1))
            y_p = ctx.enter_context(tc.tile_pool(name="y", bufs=1))
            aw_p = ctx.enter_context(tc.tile_pool(name="aw", bufs=4))
            prod_p = ctx.enter_context(tc.tile_pool(name="prod", bufs=4))
            inv_p = ctx.enter_context(tc.tile_pool(name="invs", bufs=2))
            hT_p = ctx.enter_context(tc.tile_pool(name="hT", bufs=2))
            stg_p = ctx.enter_context(tc.tile_pool(name="stg", bufs=2))
            norm_p = ctx.enter_context(tc.tile_pool(name="norm", bufs=2))
            misc_p = ctx.enter_context(tc.tile_pool(name="misc", bufs=1))
            psum_p = ctx.enter_context(
                tc.tile_pool(name="psum", bufs=8, space="PSUM"))
            dram_p = ctx.enter_context(
                tc.tile_pool(name="dram", bufs=2, space="DRAM"))

            ident = const_p.tile([P, P], FP, tag="ident")
            make_identity(nc, ident)
            ident16 = const_p.tile([P, P], F16, tag="ident16")
            nc.scalar.copy(ident16[:], ident[:])

            # --- constant tiles (loads deferred; see _load_w) ---
            wc_t, wb_t = [], []
            for l in range(3):
                ND = DINS[l] // P
                wct = const_p.tile([P, ND * DOUT], FPR, tag=f"wc{l}")
                wbt = const_p.tile([P, ND * DOUT], FPR, tag=f"wb{l}")
                wc_t.append(wct)
                wb_t.append(wbt)

            def _load_w(l):
                ND = DINS[l] // P
                for d in range(ND):
                    nc.scalar.dma_start(
                        wc_t[l][:, d * DOUT:(d + 1) * DOUT],
                        wc_d[l].ap()[d * P:(d + 1) * P, :].bitcast(FPR))
                    nc.scalar.dma_start(
                        wb_t[l][:, d * DOUT:(d + 1) * DOUT],
                        wb_d[l].ap()[d * P:(d + 1) * P, :].bitcast(FPR))

            _load_w(0)
            p1_t = const_p.tile([P, (D3 // P) * DEC], FP, tag="p1")
            p2_t = const_p.tile([P, DEC], FP, tag="p2")
            ivrT_t = const_p.tile([P, NT, 2], F16, tag="ivrT")
            nc.scalar.dma_start(
                ivrT_t[:],
                ivrT_d.ap().rearrange("(a p) c -> p a c", p=P))

            # --- h double generation (fp16) ---
            h_e = [h_p.tile([P, D3], F16, tag=f"he{k}", name="h_e")
                   for k in range(NT)]
            h_o = [h_p.tile([P, D3], F16, tag=f"ho{k}", name="h_o")
                   for k in range(NT)]
            for kt in range(NT):
                nc.sync.dma_start(h_e[kt][:, 0:DINS[0]],
                                  x_d.ap()[kt * P:(kt + 1) * P, :])

            sT = [sT_p.tile([P, N], FPR, tag=f"sT{d}", name="sT")
                  for d in range(6)]
            y_t = [y_p.tile([P, DOUT], F16, tag=f"y{j}", name="y_t")
                   for j in range(NT)]
            w_sb = misc_p.tile([P, 2 * NT], F16, tag="w_sb")

            def _dump_and_done(src_ap):
                y_sb0 = misc_p.tile([1, 1], FP, tag="y_dbg")
                nc.vector.tensor_copy(y_sb0[:], src_ap)
                nc.sync.dma_start(y_d.ap(), y_sb0[:])

            # l2 prefold bursts: split quarters between the two boundaries
            burst_q = [list(range(NQ))[:(NQ + 1) // 2],
                       list(range(NQ))[(NQ + 1) // 2:]]

            def l2_burst(qlist, hcur_unused):
                for qg in qlist:
                    pu = psum_p.tile([P, JB], FP, tag="ps", name="pu")
                    for jj in range(NT // 2):
                        aw2t = aw_p.tile([P, 2, 2 * JB], F16, tag="aw")
                        nc.sync.dma_start(
                            aw2t[:],
                            aw_d[2].ap()[jj * 2 * P:(jj + 1) * 2 * P,
                                         qg * 2 * JB:(qg + 1) * 2 * JB]
                            .rearrange("(a p) c -> p a c", p=P))
                        pr = prod_p.tile([P, 2, JB], F16, tag="prod")
                        nc.vector.tensor_tensor(pr[:], aw2t[:, :, 0:JB],
                                                aw2t[:, :, JB:2 * JB],
                                                ALU.mult)
                        for a in range(2):
                            j = jj * 2 + a
                            nc.tensor.matmul(pu[0:2, :], ivrT_t[:, j, :],
                                             pr[:, a, :], start=(j == 0),
                                             stop=(j == NT - 1))
                    usb = misc_p.tile([2, JB], FPR, tag="usb")
                    nc.scalar.copy(usb[:], pu[0:2, :])
                    for kk in range(JB // P):
                        k = qg * (JB // P) + kk
                        if k >= NT:
                            break
                        ptw = psum_p.tile([P, 2], FP, tag="ps", name="ptw")
                        nc.tensor.transpose(
                            ptw[:], usb[:, kk * P:(kk + 1) * P].bitcast(FP),
                            ident[0:2, 0:2])
                        nc.vector.tensor_copy(w_sb[:, 2 * k:2 * k + 2], ptw[:])

            if stop_phase <= 1:
                _dump_and_done(h_e[0][0:1, 0:1])

            hcur, hnxt = h_e, h_o
            n_layers = 0 if stop_phase <= 1 else (2 if stop_phase <= 6 else 2)
            for l in range(2):
                if stop_phase <= 1:
                    break
                din = DINS[l]
                ND = din // P

                # ---- mm1: sT = (prod^T @ h)^T ----
                for jb in range(NJB):
                    pss = [psum_p.tile([P, JB], FP, tag="ps", name="pmm1")
                           for _ in range(ND)]
                    for kk in range(NT // 2):
                        awt = aw_p.tile([P, 2, 2 * JB], F16, tag="aw")
                        nc.sync.dma_start(
                            awt[:],
                            aw_d[l].ap()[kk * 2 * P:(kk + 1) * 2 * P,
                                         jb * 2 * JB:(jb + 1) * 2 * JB]
                            .rearrange("(a p) c -> p a c", p=P))
                        pr = prod_p.tile([P, 2, JB], F16, tag="prod")
                        nc.vector.tensor_tensor(pr[:], awt[:, :, 0:JB],
                                                awt[:, :, JB:2 * JB],
                                                ALU.mult)
                        for a in range(2):
                            k = kk * 2 + a
                            for d in range(ND):
                                nc.tensor.matmul(
                                    pss[d][:], hcur[k][:, d * P:(d + 1) * P],
                                    pr[:, a, :], start=(k == 0),
                                    stop=(k == NT - 1))
                    for d in range(ND):
                        dst = sT[d][:, jb * JB:(jb + 1) * JB]
                        if d % 2 == 0:
                            nc.scalar.copy(dst, pss[d][:])
                        else:
                            nc.vector.tensor_copy(dst, pss[d][:])

                if stop_phase == 2 and l == 0:
                    _dump_and_done(sT[0][0:1, 0:1])
                    break

                # ---- y = s @ Wc  (fp16) ----
                for j in range(NT):
                    py = psum_p.tile([P, DOUT], FP, tag="ps", name="py")
                    for d in range(ND):
                        nc.tensor.matmul(
                            py[:], sT[d][:, j * P:(j + 1) * P],
                            wc_t[l][:, d * DOUT:(d + 1) * DOUT],
                            start=(d == 0), stop=(d == ND - 1))
                    nc.scalar.copy(y_t[j][:], py[:])

                if stop_phase == 3 and l == 0:
                    _dump_and_done(y_t[0][0:1, 0:1])
                    break

                # ---- x' = inv @ y : own cat chunk ----
                pxs = [psum_p.tile([P, JB], FP, tag="ps", name="px")
                       for _ in range((NT + 1) // 2)]
                for px in pxs:
                    nc.vector.memset(px[:], 0.0)
                for jj in range(NT // 2):
                    ivt = inv_p.tile([P, 2, N], F16, tag="inv")
                    nc.sync.dma_start(
                        ivt[:],
                        invT_d.ap()[jj * 2 * P:(jj + 1) * 2 * P, :]
                        .rearrange("(a p) c -> p a c", p=P))
                    for a in range(2):
                        j = jj * 2 + a
                        for i in range(NT):
                            px = pxs[i // 2]
                            sl = slice((i % 2) * DOUT, (i % 2 + 1) * DOUT)
                            nc.tensor.matmul(
                                px[:, sl], ivt[:, a, i * P:(i + 1) * P],
                                y_t[j][:],
                                start=False, stop=(j == NT - 1),
                                skip_group_check=True)

                if stop_phase == 4 and l == 0:
                    _dump_and_done(y_t[0][0:1, 0:1])
                    break

                # ---- stage + AllGather (two halves) ----
                stage_d = [dram_p.tile([N // 2, DOUT], F16, tag=f"stg{hh}",
                                       name="stage_d") for hh in range(2)]
                GW = min(4, HALF)  # i-blocks per staged DMA
                for g in range(NT // GW):
                    st = stg_p.tile([P, GW, DOUT], F16, tag="stg")
                    for a2 in range(GW // 2):
                        nc.scalar.copy(
                            st[:, 2 * a2:2 * a2 + 2, :],
                            pxs[(g * GW) // 2 + a2][:])
                    hh, go = divmod(g, HALF // GW)
                    nc.scalar.dma_start(
                        stage_d[hh][go * GW * P:(go + 1) * GW * P, :]
                        .rearrange("(a p) c -> p a c", p=P),
                        st[:])
                ag_t = []
                for hh in range(2):
                    agt = dram_p.tile([2, N // 2, DOUT], F16, tag=f"ag{hh}",
                                      name="ag_t")
                    nc.gpsimd.collective_compute(
                        "AllGather", ALU.bypass, replica_groups=groups,
                        ins=[stage_d[hh].opt()], outs=[agt.opt()])
                    ag_t.append(agt)

                # ---- weight prefetch for later phases (scalar queue) ----
                if l == 0:
                    _load_w(1)
                else:
                    _load_w(2)
                    for d in range(D3 // P):
                        nc.scalar.dma_start(p1_t[:, d * DEC:(d + 1) * DEC],
                                            p1_d.ap()[d * P:(d + 1) * P, :])
                    nc.scalar.dma_start(p2_t[:], p2_d.ap())

                # ---- bias = h @ Wb (full, local) -> h' cols 512:768 ----
                for it in range(NT):
                    pb = psum_p.tile([P, DOUT], FP, tag="ps", name="pb")
                    for dd in range(ND // 2):
                        ptr = psum_p.tile([P, 2 * P], F16, tag="ps",
                                          name="ptr")
                        nc.tensor.matmul(
                            ptr[:, 0:P], hcur[it][:, 2 * dd * P:
                                                  (2 * dd + 1) * P],
                            ident16[:], is_transpose=True,
                            start=True, stop=False, skip_group_check=True)
                        nc.tensor.matmul(
                            ptr[:, P:2 * P],
                            hcur[it][:, (2 * dd + 1) * P:(2 * dd + 2) * P],
                            ident16[:], is_transpose=True,
                            start=False, stop=True, skip_group_check=True)
                        hTt = hT_p.tile([P, 2 * P], FPR, tag="hT")
                        nc.vector.tensor_copy(hTt[:], ptr[:])
                        for a in range(2):
                            d = 2 * dd + a
                            nc.tensor.matmul(
                                pb[:], hTt[:, a * P:(a + 1) * P],
                                wb_t[l][:, d * DOUT:(d + 1) * DOUT],
                                start=(d == 0), stop=(d == ND - 1))
                    nc.scalar.copy(hnxt[it][:, 2 * DOUT:3 * DOUT], pb[:])

                # ---- l2 prefold burst (fills AG latency) ----
                l2_burst(burst_q[l], hcur)

                # ---- assemble + normalize + lrelu -> h' ----
                for it in range(NT):
                    hh, io = divmod(it, HALF)
                    ag = ag_t[hh]
                    ht = hnxt[it]
                    nc.sync.dma_start(
                        ht[:, 0:2 * DOUT],
                        ag[:, io * P:(io + 1) * P, :]
                        .rearrange("r p c -> p r c"))
                    if stop_phase == 4.75 and l == 0:
                        dbf = misc_p.tile([P, D3], FP, tag="dbf", bufs=2)
                        nc.vector.tensor_copy(dbf[:], ht[:])
                        nc.sync.dma_start(
                            dbg_d.ap()[it * P:(it + 1) * P, :], dbf[:])
                        continue
                    sqs = norm_p.tile([P, D3], F16, tag="sq")
                    ssq = norm_p.tile([P, 1], FP, tag="ssq")
                    if it % 2 == 0:
                        nc.scalar.activation(sqs[:], ht[:], AF.Square,
                                             accum_out=ssq[:])
                    else:
                        nc.vector.scalar_tensor_tensor(
                            sqs[:], ht[:], 1.0, ht[:], ALU.mult, ALU.mult,
                            accum_out=ssq[:])
                    nrm = norm_p.tile([P, 1], FP, tag="nrm")
                    nc.scalar.activation(nrm[:], ssq[:], AF.Sqrt)
                    nc.vector.tensor_scalar_max(nrm[:], nrm[:], EPS)
                    rn = norm_p.tile([P, 1], FP, tag="rn")
                    nc.vector.reciprocal(rn[:], nrm[:])
                    nc.scalar.mul(ht[:], ht[:], rn[:])
                    nc.vector.scalar_tensor_tensor(
                        ht[:], ht[:], LEAK, ht[:], ALU.mult, ALU.max)

                if stop_phase == 4.75 and l == 0:
                    break
                hcur, hnxt = hnxt, hcur
                if stop_phase == 5 and l == 0:
                    _dump_and_done(hcur[0][0:1, 0:1])
                    for it in range(NT):
                        dbf = misc_p.tile([P, D3], FP, tag="dbf", bufs=2)
                        nc.vector.tensor_copy(dbf[:], hcur[it][:])
                        nc.sync.dma_start(
                            dbg_d.ap()[it * P:(it + 1) * P, :], dbf[:])
                    break

            do_tail = stop_phase > 6
            if do_tail:
                # ---- l2: x2 = w^T @ h ----
                psA = psum_p.tile([P, JB], FP, tag="ps", name="psA")
                psB = psum_p.tile([P, JB], FP, tag="ps", name="psB")
                for k in range(NT):
                    nc.tensor.matmul(psA[0:2, :], w_sb[:, 2 * k:2 * k + 2],
                                     hcur[k][:, 0:JB], start=(k == 0),
                                     stop=(k == NT - 1))
                    nc.tensor.matmul(psB[0:2, 0:D3 - JB],
                                     w_sb[:, 2 * k:2 * k + 2],
                                     hcur[k][:, JB:D3], start=(k == 0),
                                     stop=(k == NT - 1))
                x2sb = misc_p.tile([2, D3], FPR, tag="x2sb")
                nc.scalar.copy(x2sb[:, 0:JB], psA[0:2, :])
                nc.scalar.copy(x2sb[:, JB:D3], psB[0:2, 0:D3 - JB])

                # own2 = x2 @ Wc2
                pc2 = psum_p.tile([P, DOUT], FP, tag="ps", name="pc2")
                for d in range(D3 // P):
                    ptx = psum_p.tile([P, 2], FP, tag="ps", name="ptx")
                    nc.tensor.transpose(
                        ptx[:], x2sb[:, d * P:(d + 1) * P].bitcast(FP),
                        ident[0:2, 0:2])
                    x2T = hT_p.tile([P, 2], FPR, tag="x2T")
                    nc.vector.tensor_copy(x2T[:], ptx[:])
                    nc.tensor.matmul(pc2[0:2, :], x2T[:],
                                     wc_t[2][:, d * DOUT:(d + 1) * DOUT],
                                     start=(d == 0), stop=(d == D3 // P - 1))

                # bias2 = hdrug @ Wb2 (hdrug via DRAM bounce, post-norm rows)
                bounce = dram_p.tile([2, D3], F16, tag="bounce")
                nc.sync.dma_start(bounce[:], hcur[NT - 1][P - 2:P, :])
                hdr = misc_p.tile([2, D3], F16, tag="hdrug")
                nc.sync.dma_start(hdr[:], bounce[:])
                pb2 = psum_p.tile([P, DOUT], FP, tag="ps", name="pb2")
                for d in range(D3 // P):
                    ptr = psum_p.tile([P, 2], F16, tag="ps", name="ptr2")
                    nc.tensor.transpose(ptr[:], hdr[:, d * P:(d + 1) * P],
                                        ident16[0:2, 0:2])
                    hTt = hT_p.tile([P, 2], FPR, tag="x2T")
                    nc.vector.tensor_copy(hTt[:], ptr[:])
                    nc.tensor.matmul(pb2[0:2, :], hTt[:],
                                     wb_t[2][:, d * DOUT:(d + 1) * DOUT],
                                     start=(d == 0), stop=(d == D3 // P - 1))

                # stage2 + AG + assemble + norm
                stg2 = stg_p.tile([2, DOUT], F16, tag="stg")
                nc.scalar.copy(stg2[:], pc2[0:2, :])
                st2d = dram_p.tile([2, DOUT], F16, tag="stg2")
                nc.scalar.dma_start(st2d[:], stg2[:])
                ag2 = dram_p.tile([2, 2, DOUT], F16, tag="ag2")
                nc.gpsimd.collective_compute(
                    "AllGather", ALU.bypass, replica_groups=groups,
                    ins=[st2d.opt()], outs=[ag2.opt()])
                asm2 = stg_p.tile([2, 2 * DOUT], F16, tag="asm2")
                nc.sync.dma_start(asm2[:, 0:DOUT], ag2[0, :, :])
                nc.sync.dma_start(asm2[:, DOUT:2 * DOUT], ag2[1, :, :])
                dr = misc_p.tile([2, D3], FP, tag="drug")
                nc.vector.tensor_copy(dr[:, 0:2 * DOUT], asm2[:])
                nc.vector.tensor_copy(dr[:, 2 * DOUT:D3], pb2[0:2, :])
                sq = norm_p.tile([2, D3], FP, tag="sq2")
                ssq = norm_p.tile([2, 1], FP, tag="ssq2")
                nc.vector.tensor_tensor(sq[:], dr[:], dr[:], ALU.mult)
                nc.vector.tensor_reduce(ssq[:], sq[:], AxisListType.X,
                                        ALU.add)
                nrm = norm_p.tile([2, 1], FP, tag="nrm2")
                nc.scalar.activation(nrm[:], ssq[:], AF.Sqrt)
                nc.vector.tensor_scalar_max(nrm[:], nrm[:], EPS)
                rn = norm_p.tile([2, 1], FP, tag="rn2")
                nc.vector.reciprocal(rn[:], nrm[:])
                nc.vector.tensor_scalar(dr[:], dr[:], rn[:], None, ALU.mult)
                nc.scalar.mul(sq[:], dr[:], LEAK)
                nc.vector.tensor_max(dr[:], dr[:], sq[:])

                # ---- head: ypred = (a P1 P2) . (b P1) ----
                ND3 = D3 // P
                dT = misc_p.tile([P, ND3 * 2], FP, tag="dT")
                for d in range(ND3):
                    pt = psum_p.tile([P, 2], FP, tag="ps", name="phd")
                    nc.tensor.transpose(pt[:], dr[:, d * P:(d + 1) * P],
                                        ident[0:2, 0:2])
                    nc.vector.tensor_copy(dT[:, d * 2:(d + 1) * 2], pt[:])
                pw = psum_p.tile([P, 2], FP, tag="ps", name="pw")
                for d in range(ND3):
                    nc.tensor.matmul(pw[:], p1_t[:, d * DEC:(d + 1) * DEC],
                                     dT[:, d * 2:(d + 1) * 2],
                                     start=(d == 0), stop=(d == ND3 - 1))
                w_hd = misc_p.tile([P, 2], FP, tag="w_hd")
                nc.vector.tensor_copy(w_hd[:], pw[:])
                ptt = psum_p.tile([P, 1], FP, tag="ps", name="ptt")
                nc.tensor.matmul(ptt[:], p2_t[:], w_hd[:, 0:1], start=True,
                                 stop=True)
                t_sb = misc_p.tile([P, 1], FP, tag="t_sb")
                nc.vector.tensor_copy(t_sb[:], ptt[:])
                py_ = psum_p.tile([1, 1], FP, tag="ps", name="pyf")
                nc.tensor.matmul(py_[:], t_sb[:], w_hd[:, 1:2], start=True,
                                 stop=True)
                y_sb = misc_p.tile([1, 1], FP, tag="y_sb")
                nc.vector.tensor_copy(y_sb[:], py_[:])
                nc.sync.dma_start(y_d.ap(), y_sb[:])
            elif stop_phase > 5:
                _dump_and_done(hcur[0][0:1, 0:1])

    nc.compile()
    return nc


# ---------------------------------------------------------------------------
# Host-side input prep
# ---------------------------------------------------------------------------

def _pack_quarters(a16, w16, N):
    """Interleave 512-col quarters: [a[:,q] | w[:,q]] -> [N, 2N] fp16."""
    NQ = max(N // JB, 1)
    q = min(JB, N)
    out = np.empty((N, 2 * N), dtype=np.float16)
    for i in range(NQ):
        out[:, i * 2 * q:i * 2 * q + q] = a16[:, i * q:(i + 1) * q]
        out[:, i * 2 * q + q:(i + 1) * 2 * q] = w16[:, i * q:(i + 1) * q]
    return out


def make_in_maps(inputs: dict, n_cores: int, N: int = None):
    """Per-core input dicts. Core 2b = up path of batch b, 2b+1 = down."""
    if N is None:
        N = np.asarray(inputs["adj"]).shape[-1]
    f32c = lambda a: np.ascontiguousarray(np.asarray(a, dtype=np.float32))
    f16c = lambda a: np.ascontiguousarray(np.asarray(a).astype(np.float16))

    def bake_mask(w):
        w = np.array(w, dtype=np.float32)
        w[-2:, :] = 1.0
        w[:, -2:] = 1.0
        return w

    maps = []
    for c in range(n_cores):
        b, down = divmod(c, 2)
        if not down:
            A = np.asarray(inputs["adj"][b])
            IV = np.asarray(inputs["up_inv_deg"][b])
            was = [bake_mask(inputs[f"l{l}_up_adj_w"]) for l in range(3)]
            wcs = [inputs[f"l{l}_up_w"] for l in range(3)]
        else:
            A = np.asarray(inputs["adj"][b]).T
            IV = np.asarray(inputs["down_inv_deg"][b])
            was = [bake_mask(inputs[f"l{l}_down_adj_w"]).T for l in range(3)]
            wcs = [inputs[f"l{l}_down_w"] for l in range(3)]
        A16 = f16c(A)
        m = {
            "x": f16c(inputs["x"][b]),
            "invT": f16c(IV.T),
            "ivrT": f16c(IV[-2:, :].T),
            "p1": f32c(inputs["parameter1"]),
            "p2": f32c(inputs["parameter2"]),
        }
        for l in range(2):
            m[f"aw{l}"] = _pack_quarters(A16, f16c(was[l]), N)
        # l2: prodT = adjT .* waT (transpose of this core's own matrices)
        m["aw2"] = _pack_quarters(
            np.ascontiguousarray(A16.T), f16c(was[2].T), N)
        for l in range(3):
            m[f"w{l}c"] = f32c(wcs[l])
            m[f"w{l}b"] = f32c(inputs[f"l{l}_bias"])
        maps.append(m)
    return maps


_nc_cache = {}


def _get_program(n_cores, N):
    key = (n_cores, N)
    if key not in _nc_cache:
        _nc_cache[key] = build_program(n_cores, N)
    return _nc_cache[key]


def kernel(**inputs) -> np.ndarray:
    n_cores = 8
    nc = _get_program(n_cores, N_FULL)
    in_maps = make_in_maps(inputs, n_cores)
    res = run_bass_kernel_spmd(nc, in_maps, core_ids=list(range(n_cores)))
    out = np.zeros((B, 1), dtype=np.float32)
    for b in range(B):
        out[b, 0] = res.results[2 * b]["ypred"][0, 0]
    return out


# revision 33
# speedup vs baseline: 2.3383x; 1.0819x over previous
"""BiGraphSAGEDecoder Trainium2 kernel (v2 — restructured).

Sharding: 8 cores = 4 batches x {up-path, down-path}. One SPMD bass program;
up/down asymmetry handled by data (down cores get host-transposed matrices).

Key restructurings vs v1:
  - mm1 emits s^T directly (lhsT = h blocks, rhs = prod strips), enabling
    the reassociation x' = inv @ (s @ Wc): the big [N,N] matmul contracts
    against 256 cols instead of din=768.
  - layer 2 never computes s: x2 = ivr @ prod^T @ h is folded as
    u = ivr @ prodT (prodT streamed from host-transposed adj/wadj),
    w = u^T, x2 = w^T @ h.
  - full bias chunk computed locally (no bias in the exchange); AllGather
    moves only the 256-col own chunk per rank, in fp16.
  - dtypes: adj/wadj/inv fp16 in DRAM, prod fp16, h fp16, y fp16,
    sT fp32r, weights fp32r, psum fp32. Validated max rel err 1.9e-3
    vs fp32 reference (tolerance 2e-2).

Math per layer (per core, its path):
  prod = adj .* wadj_baked                  (DVE, fp16)
  sT   = (prod^T @ h)^T  via lhsT=h         (PE)
  y    = s @ Wc                             (PE, lhsT = sT blocks)
  own  = inv @ y                            (PE, lhsT = invT blocks)
  bias = h @ Wb (full)                      (PE, via PE-transposed h)
  exchange own chunks (2-rank AllGather, two halves, fp16)
  h' = lrelu(cat / max(||cat||, eps))       (DVE/scalar/gpsimd)
"""

import os
import sys
import types
import contextlib

sys.path.insert(0, "/opt/trn_rl_repo")

import numpy as np

import concourse.bass as bass
import concourse.tile as tile
from concourse import mybir, bacc
from concourse.mybir import AxisListType
from concourse.masks import make_identity
from concourse.bass_utils import run_bass_kernel_spmd

FP = mybir.dt.float32
FPR = mybir.dt.float32r
F16 = mybir.dt.float16
AF = mybir.ActivationFunctionType
ALU = mybir.AluOpType

# ---------------------------------------------------------------------------
# Environment patches (required for this container's toolchain)
# ---------------------------------------------------------------------------


def install_ntff_shim():
    """antenv.axon_hooks is absent in this image; provide it so trace=True
    profiling works (used by test.py, harmless otherwise)."""
    try:
        import antenv.axon_hooks  # noqa: F401
        return
    except ImportError:
        pass
    try:
        import antenv
    except ImportError:
        return
    mod = types.ModuleType("antenv.axon_hooks")
    _holder = {"hook": None}
    mod.set_axon_ntff_profile_hook = lambda h: _holder.__setitem__("hook", h)
    mod.get_axon_ntff_profile_hook = lambda: _holder["hook"]
    sys.modules["antenv.axon_hooks"] = mod
    antenv.axon_hooks = mod
    try:
        from trn_agent_boot.trn_boot import _ntff_profile_via_ctypes

        hook = _ntff_profile_via_ctypes("/opt/axon/libaxon_pjrt.so")
        if hook is not None:
            mod.set_axon_ntff_profile_hook(hook)
    except Exception:
        pass


install_ntff_shim()

if os.environ.get("KGSD_LDW_OPT", "0") != "0":
    # let walrus dedup back-to-back LDWEIGHTS
    import concourse.bass_utils as _bu
    _orig_run_command = _bu.run_command

    def _patched_run_command(argv, **kw):
        argv = ["--enable-ldw-opt=true" if a == "--enable-ldw-opt=false"
                else a for a in argv]
        return _orig_run_command(argv, **kw)

    _bu.run_command = _patched_run_command

# ---------------------------------------------------------------------------
# Problem constants
# ---------------------------------------------------------------------------

N_FULL = 2048
B = 4
P = 128
DOUT = 256     # per-path cat chunk width (also bias width)
DEC = 128
DINS = (256, 768, 768)   # per-layer input dims
D3 = 768
EPS = 1e-12
LEAK = 0.1
JB = 512       # mm1 / x' column block (one PSUM bank of fp32)


# ---------------------------------------------------------------------------
# Program builder
# ---------------------------------------------------------------------------


def build_program(n_cores: int, N: int = N_FULL, stop_phase: int = 99):
    """Build the SPMD bass program.

    stop_phase (debug): 1=x load, 2=l0 mm1, 3=l0 y, 4=l0 x', 5=l0 done,
    6=l1 done, 7=full.
    """
    NT = N // P              # 128-row tiles
    NJB = N // JB            # mm1/x' column blocks
    HALF = NT // 2
    NQ = max(N // JB, 1)     # aw quarter count (512-wide pairs)

    nc = bacc.Bacc("TRN2", target_bir_lowering=False, debug=False,
                   num_devices=n_cores)

    # --- DRAM I/O ---
    x_d = nc.dram_tensor("x", [N, DINS[0]], F16, kind="ExternalInput")
    adj_d = nc.dram_tensor("adj", [N, N], F16, kind="ExternalInput")
    wa_d = [nc.dram_tensor(f"wa{l}", [N, N], F16, kind="ExternalInput")
            for l in range(2)]
    # aw2: interleaved quarters [adjT[:,q*512:(q+1)*512] | waT[:,q*512:...]]
    aw2_d = nc.dram_tensor("aw2", [N, 2 * N], F16, kind="ExternalInput")
    invT_d = nc.dram_tensor("invT", [N, N], F16, kind="ExternalInput")
    ivrT_d = nc.dram_tensor("ivrT", [N, 2], F16, kind="ExternalInput")
    wc_d = [nc.dram_tensor(f"w{l}c", [DINS[l], DOUT], FP, kind="ExternalInput")
            for l in range(3)]
    wb_d = [nc.dram_tensor(f"w{l}b", [DINS[l], DOUT], FP, kind="ExternalInput")
            for l in range(3)]
    p1_d = nc.dram_tensor("p1", [D3, DEC], FP, kind="ExternalInput")
    p2_d = nc.dram_tensor("p2", [DEC, DEC], FP, kind="ExternalInput")
    y_d = nc.dram_tensor("ypred", [1, 1], FP, kind="ExternalOutput")
    dbg_d = (nc.dram_tensor("dbg", [N, D3], FP, kind="ExternalOutput")
             if stop_phase < 99 else None)

    groups = [[i, i + 1] for i in range(0, n_cores, 2)]

    with tile.TileContext(nc) as tc:
        with contextlib.ExitStack() as ctx:
            const_p = ctx.enter_context(tc.tile_pool(name="const", bufs=1))
            h_p = ctx.enter_context(tc.tile_pool(name="h", bufs=1))
            sT_p = ctx.enter_context(tc.tile_pool(name="sT", bufs=1))
            y_p = ctx.enter_context(tc.tile_pool(name="y", bufs=1))
            aw_p = ctx.enter_context(tc.tile_pool(name="aw", bufs=4))
            prod_p = ctx.enter_context(tc.tile_pool(name="prod", bufs=4))
            inv_p = ctx.enter_context(tc.tile_pool(name="invs", bufs=2))
            hT_p = ctx.enter_context(tc.tile_pool(name="hT", bufs=2))
            stg_p = ctx.enter_context(tc.tile_pool(name="stg", bufs=2))
            norm_p = ctx.enter_context(tc.tile_pool(name="norm", bufs=2))
            misc_p = ctx.enter_context(tc.tile_pool(name="misc", bufs=1))
            psum_p = ctx.enter_context(
                tc.tile_pool(name="psum", bufs=8, space="PSUM"))
            dram_p = ctx.enter_context(
                tc.tile_pool(name="dram", bufs=2, space="DRAM"))

            ident = const_p.tile([P, P], FP, tag="ident")
            make_identity(nc, ident)
            ident16 = const_p.tile([P, P], F16, tag="ident16")
            nc.scalar.copy(ident16[:], ident[:])

            # --- constant tiles (loads deferred; see _load_w) ---
            wc_t, wb_t = [], []
            for l in range(3):
                ND = DINS[l] // P
                wct = const_p.tile([P, ND * DOUT], FPR, tag=f"wc{l}")
                wbt = const_p.tile([P, ND * DOUT], FPR, tag=f"wb{l}")
                wc_t.append(wct)
                wb_t.append(wbt)

            def _load_w(l):
                ND = DINS[l] // P
                for d in range(ND):
                    nc.scalar.dma_start(
                        wc_t[l][:, d * DOUT:(d + 1) * DOUT],
                        wc_d[l].ap()[d * P:(d + 1) * P, :].bitcast(FPR))
                    nc.scalar.dma_start(
                        wb_t[l][:, d * DOUT:(d + 1) * DOUT],
                        wb_d[l].ap()[d * P:(d + 1) * P, :].bitcast(FPR))

            _load_w(0)
            p1_t = const_p.tile([P, (D3 // P) * DEC], FP, tag="p1")
            p2_t = const_p.tile([P, DEC], FP, tag="p2")
            ivrT_t = const_p.tile([P, NT, 2], F16, tag="ivrT")
            nc.scalar.dma_start(
                ivrT_t[:],
                ivrT_d.ap().rearrange("(a p) c -> p a c", p=P))

            # --- h double generation (fp16) ---
            h_e = [h_p.tile([P, D3], F16, tag=f"he{k}", name="h_e")
                   for k in range(NT)]
            h_o = [h_p.tile([P, D3], F16, tag=f"ho{k}", name="h_o")
                   for k in range(NT)]
            # x loads are interleaved with the first mm1 strip loads below

            sT = [sT_p.tile([P, N], FPR, tag=f"sT{d}", name="sT")
                  for d in range(6)]
            y_t = [y_p.tile([P, DOUT], F16, tag=f"y{j}", name="y_t")
                   for j in range(NT)]
            w_sb = misc_p.tile([P, 2 * NT], F16, tag="w_sb")

            def _dump_and_done(src_ap):
                y_sb0 = misc_p.tile([1, 1], FP, tag="y_dbg")
                nc.vector.tensor_copy(y_sb0[:], src_ap)
                nc.sync.dma_start(y_d.ap(), y_sb0[:])

            # l2 prefold bursts: split quarters between the two boundaries;
            # steps are interleaved into the bias loops (one step per it)
            burst_q = [list(range(NQ))[:(NQ + 1) // 2],
                       list(range(NQ))[(NQ + 1) // 2:]]
            pu_live = {}

            def burst_step(qg, jj):
                if jj == 0:
                    pu_live[qg] = psum_p.tile([P, JB], FP, tag="ps",
                                              name="pu")
                pu = pu_live[qg]
                aw2t = aw_p.tile([P, 2, 2 * JB], F16, tag="aw", bufs=2)
                nc.sync.dma_start(
                    aw2t[:],
                    aw2_d.ap()[jj * 2 * P:(jj + 1) * 2 * P,
                               qg * 2 * JB:(qg + 1) * 2 * JB]
                    .rearrange("(a p) c -> p a c", p=P))
                pr = prod_p.tile([P, 2, JB], F16, tag="prod")
                nc.vector.tensor_tensor(pr[:], aw2t[:, :, 0:JB],
                                        aw2t[:, :, JB:2 * JB], ALU.mult)
                for a in range(2):
                    j = jj * 2 + a
                    nc.tensor.matmul(pu[0:2, :], ivrT_t[:, j, :],
                                     pr[:, a, :], start=(j == 0),
                                     stop=(j == NT - 1))
                if jj == NT // 2 - 1:
                    usb = misc_p.tile([2, JB], F16, tag="usb")
                    nc.scalar.copy(usb[:], pu[0:2, :])
                    for kk in range(JB // P):
                        k = qg * (JB // P) + kk
                        if k >= NT:
                            break
                        ptw = psum_p.tile([P, 2], F16, tag="ps", name="ptw")
                        nc.tensor.transpose(
                            ptw[:], usb[:, kk * P:(kk + 1) * P],
                            ident16[0:2, 0:2])
                        nc.vector.tensor_copy(w_sb[:, 2 * k:2 * k + 2],
                                              ptw[:])

            if stop_phase <= 1:
                _dump_and_done(h_e[0][0:1, 0:1])

            hcur, hnxt = h_e, h_o
            n_layers = 0 if stop_phase <= 1 else (2 if stop_phase <= 6 else 2)
            for l in range(2):
                if stop_phase <= 1:
                    break
                din = DINS[l]
                ND = din // P

                # ---- mm1: sT = (prod^T @ h)^T ----
                for jb in range(NJB):
                    pss = [psum_p.tile([P, JB], FP, tag="ps", name="pmm1")
                           for _ in range(ND)]
                    for kk in range(NT // 2):
                        if l == 0 and jb == 0:
                            for a in range(2):
                                kt = kk * 2 + a
                                nc.sync.dma_start(
                                    h_e[kt][:, 0:DINS[0]],
                                    x_d.ap()[kt * P:(kt + 1) * P, :])
                        rsl = slice(kk * 2 * P, (kk + 1) * 2 * P)
                        csl = slice(jb * JB, (jb + 1) * JB)
                        adt = aw_p.tile([P, 2, JB], F16, tag="adj")
                        nc.sync.dma_start(
                            adt[:], adj_d.ap()[rsl, csl]
                            .rearrange("(a p) c -> p a c", p=P))
                        wat = aw_p.tile([P, 2, JB], F16, tag="wa")
                        nc.sync.dma_start(
                            wat[:], wa_d[l].ap()[rsl, csl]
                            .rearrange("(a p) c -> p a c", p=P))
                        pr = prod_p.tile([P, 2, JB], F16, tag="prod")
                        nc.vector.tensor_tensor(pr[:], adt[:], wat[:],
                                                ALU.mult)
                        for a in range(2):
                            k = kk * 2 + a
                            for d in range(ND):
                                nc.tensor.matmul(
                                    pss[d][:], hcur[k][:, d * P:(d + 1) * P],
                                    pr[:, a, :], start=(k == 0),
                                    stop=(k == NT - 1))
                    for d in range(ND):
                        dst = sT[d][:, jb * JB:(jb + 1) * JB]
                        if d % 2 == 0:
                            nc.scalar.copy(dst, pss[d][:])
                        else:
                            nc.vector.tensor_copy(dst, pss[d][:])

                if stop_phase == 2 and l == 0:
                    _dump_and_done(sT[0][0:1, 0:1])
                    break

                # ---- y = s @ Wc  (fp16) ----
                for j in range(NT):
                    py = psum_p.tile([P, DOUT], FP, tag="ps", name="py")
                    for d in range(ND):
                        nc.tensor.matmul(
                            py[:], sT[d][:, j * P:(j + 1) * P],
                            wc_t[l][:, d * DOUT:(d + 1) * DOUT],
                            start=(d == 0), stop=(d == ND - 1))
                    nc.scalar.copy(y_t[j][:], py[:])

                if stop_phase == 3 and l == 0:
                    _dump_and_done(y_t[0][0:1, 0:1])
                    break

                # ---- x' = inv @ y : own cat chunk ----
                pxs = [psum_p.tile([P, JB], FP, tag="ps", name="px")
                       for _ in range((NT + 1) // 2)]
                for px in pxs:
                    nc.vector.memset(px[:], 0.0)
                for j in range(NT):
                    ivt = inv_p.tile([P, N], F16, tag="inv", bufs=3)
                    nc.sync.dma_start(ivt[:],
                                      invT_d.ap()[j * P:(j + 1) * P, :])
                    for i in range(NT):
                        px = pxs[i // 2]
                        sl = slice((i % 2) * DOUT, (i % 2 + 1) * DOUT)
                        nc.tensor.matmul(
                            px[:, sl], ivt[:, i * P:(i + 1) * P], y_t[j][:],
                            start=False, stop=(j == NT - 1),
                            skip_group_check=True)

                if stop_phase == 4 and l == 0:
                    _dump_and_done(y_t[0][0:1, 0:1])
                    break

                # ---- stage + AllGather (two halves) ----
                stage_d = [dram_p.tile([N // 2, DOUT], F16, tag=f"stg{hh}",
                                       name="stage_d") for hh in range(2)]
                GW = min(4, HALF)  # i-blocks per staged DMA
                for g in range(NT // GW):
                    st = stg_p.tile([P, GW, DOUT], F16, tag="stg")
                    for a2 in range(GW // 2):
                        nc.scalar.copy(
                            st[:, 2 * a2:2 * a2 + 2, :],
                            pxs[(g * GW) // 2 + a2][:])
                    hh, go = divmod(g, HALF // GW)
                    nc.scalar.dma_start(
                        stage_d[hh][go * GW * P:(go + 1) * GW * P, :]
                        .rearrange("(a p) c -> p a c", p=P),
                        st[:])
                ag_t = []
                for hh in range(2):
                    agt = dram_p.tile([2, N // 2, DOUT], F16, tag=f"ag{hh}",
                                      name="ag_t")
                    nc.gpsimd.collective_compute(
                        "AllGather", ALU.bypass, replica_groups=groups,
                        ins=[stage_d[hh].opt()], outs=[agt.opt()])
                    ag_t.append(agt)

                # ---- weight prefetch for later phases (scalar queue) ----
                if l == 0:
                    _load_w(1)
                else:
                    _load_w(2)
                    for d in range(D3 // P):
                        nc.scalar.dma_start(p1_t[:, d * DEC:(d + 1) * DEC],
                                            p1_d.ap()[d * P:(d + 1) * P, :])
                    nc.scalar.dma_start(p2_t[:], p2_d.ap())

                # ---- bias = h @ Wb (full, local) -> h' cols 512:768 ----
                # (one l2-prefold burst step interleaved per it)
                bsteps = [(qg, jj) for qg in burst_q[l]
                          for jj in range(NT // 2)]
                for it in range(NT):
                    if it < len(bsteps):
                        burst_step(*bsteps[it])
                    pb = psum_p.tile([P, DOUT], FP, tag="ps", name="pb")
                    for dd in range(ND // 2):
                        ptr = psum_p.tile([P, 2 * P], F16, tag="ps",
                                          name="ptr")
                        nc.tensor.matmul(
                            ptr[:, 0:P], hcur[it][:, 2 * dd * P:
                                                  (2 * dd + 1) * P],
                            ident16[:], is_transpose=True,
                            start=True, stop=False, skip_group_check=True)
                        nc.tensor.matmul(
                            ptr[:, P:2 * P],
                            hcur[it][:, (2 * dd + 1) * P:(2 * dd + 2) * P],
                            ident16[:], is_transpose=True,
                            start=False, stop=True, skip_group_check=True)
                        hTt = hT_p.tile([P, 2 * P], FPR, tag="hT")
                        nc.vector.tensor_copy(hTt[:], ptr[:])
                        for a in range(2):
                            d = 2 * dd + a
                            nc.tensor.matmul(
                                pb[:], hTt[:, a * P:(a + 1) * P],
                                wb_t[l][:, d * DOUT:(d + 1) * DOUT],
                                start=(d == 0), stop=(d == ND - 1))
                    nc.scalar.copy(hnxt[it][:, 2 * DOUT:3 * DOUT], pb[:])

                # ---- leftover burst steps (if NT < steps) ----
                for st_ in bsteps[NT:]:
                    burst_step(*st_)

                # ---- assemble + normalize + lrelu -> h' ----
                for it in range(NT):
                    hh, io = divmod(it, HALF)
                    ag = ag_t[hh]
                    ht = hnxt[it]
                    nc.sync.dma_start(
                        ht[:, 0:2 * DOUT],
                        ag[:, io * P:(io + 1) * P, :]
                        .rearrange("r p c -> p r c"))
                    if stop_phase == 4.75 and l == 0:
                        dbf = misc_p.tile([P, D3], FP, tag="dbf", bufs=2)
                        nc.vector.tensor_copy(dbf[:], ht[:])
                        nc.sync.dma_start(
                            dbg_d.ap()[it * P:(it + 1) * P, :], dbf[:])
                        continue
                    sqs = norm_p.tile([P, D3], F16, tag="sq")
                    ssq = norm_p.tile([P, 1], FP, tag="ssq")
                    if it % 2 == 0:
                        nc.scalar.activation(sqs[:], ht[:], AF.Square,
                                             accum_out=ssq[:])
                    else:
                        nc.vector.scalar_tensor_tensor(
                            sqs[:], ht[:], 1.0, ht[:], ALU.mult, ALU.mult,
                            accum_out=ssq[:])
                    nrm = norm_p.tile([P, 1], FP, tag="nrm")
                    nc.scalar.activation(nrm[:], ssq[:], AF.Sqrt)
                    nc.vector.tensor_scalar_max(nrm[:], nrm[:], EPS)
                    rn = norm_p.tile([P, 1], FP, tag="rn")
                    nc.vector.reciprocal(rn[:], nrm[:])
                    nc.scalar.mul(ht[:], ht[:], rn[:])
                    nc.vector.scalar_tensor_tensor(
                        ht[:], ht[:], LEAK, ht[:], ALU.mult, ALU.max)

                if stop_phase == 4.75 and l == 0:
                    break
                hcur, hnxt = hnxt, hcur
                if stop_phase == 5 and l == 0:
                    _dump_and_done(hcur[0][0:1, 0:1])
                    for it in range(NT):
                        dbf = misc_p.tile([P, D3], FP, tag="dbf", bufs=2)
                        nc.vector.tensor_copy(dbf[:], hcur[it][:])
                        nc.sync.dma_start(
                            dbg_d.ap()[it * P:(it + 1) * P, :], dbf[:])
                    break

            do_tail = stop_phase > 6
            if do_tail:
                # ---- l2: x2 = w^T @ h ----
                psA = psum_p.tile([P, JB], FP, tag="ps", name="psA")
                psB = psum_p.tile([P, JB], FP, tag="ps", name="psB")
                for k in range(NT):
                    nc.tensor.matmul(psA[0:2, :], w_sb[:, 2 * k:2 * k + 2],
                                     hcur[k][:, 0:JB], start=(k == 0),
                                     stop=(k == NT - 1))
                    nc.tensor.matmul(psB[0:2, 0:D3 - JB],
                                     w_sb[:, 2 * k:2 * k + 2],
                                     hcur[k][:, JB:D3], start=(k == 0),
                                     stop=(k == NT - 1))
                x2sb = misc_p.tile([2, D3], F16, tag="x2sb")
                nc.scalar.copy(x2sb[:, 0:JB], psA[0:2, :])
                nc.scalar.copy(x2sb[:, JB:D3], psB[0:2, 0:D3 - JB])

                # own2 = x2 @ Wc2
                pc2 = psum_p.tile([P, DOUT], FP, tag="ps", name="pc2")
                for d in range(D3 // P):
                    ptx = psum_p.tile([P, 2], F16, tag="ps", name="ptx")
                    nc.tensor.transpose(
                        ptx[:], x2sb[:, d * P:(d + 1) * P],
                        ident16[0:2, 0:2])
                    x2T = hT_p.tile([P, 2], FPR, tag="x2T")
                    nc.vector.tensor_copy(x2T[:], ptx[:])
                    nc.tensor.matmul(pc2[0:2, :], x2T[:],
                                     wc_t[2][:, d * DOUT:(d + 1) * DOUT],
                                     start=(d == 0), stop=(d == D3 // P - 1))

                # bias2 = hdrug @ Wb2 (hdrug via DRAM bounce, post-norm rows)
                bounce = dram_p.tile([2, D3], F16, tag="bounce")
                nc.sync.dma_start(bounce[:], hcur[NT - 1][P - 2:P, :])
                hdr = misc_p.tile([2, D3], F16, tag="hdrug")
                nc.sync.dma_start(hdr[:], bounce[:])
                pb2 = psum_p.tile([P, DOUT], FP, tag="ps", name="pb2")
                for d in range(D3 // P):
                    ptr = psum_p.tile([P, 2], F16, tag="ps", name="ptr2")
                    nc.tensor.transpose(ptr[:], hdr[:, d * P:(d + 1) * P],
                                        ident16[0:2, 0:2])
                    hTt = hT_p.tile([P, 2], FPR, tag="x2T")
                    nc.vector.tensor_copy(hTt[:], ptr[:])
                    nc.tensor.matmul(pb2[0:2, :], hTt[:],
                                     wb_t[2][:, d * DOUT:(d + 1) * DOUT],
                                     start=(d == 0), stop=(d == D3 // P - 1))

                # stage2 + AG + assemble + norm
                stg2 = stg_p.tile([2, DOUT], F16, tag="stg")
                nc.scalar.copy(stg2[:], pc2[0:2, :])
                st2d = dram_p.tile([2, DOUT], F16, tag="stg2")
                nc.scalar.dma_start(st2d[:], stg2[:])
                ag2 = dram_p.tile([2, 2, DOUT], F16, tag="ag2")
                nc.gpsimd.collective_compute(
                    "AllGather", ALU.bypass, replica_groups=groups,
                    ins=[st2d.opt()], outs=[ag2.opt()])
                asm2 = stg_p.tile([2, 2 * DOUT], F16, tag="asm2")
                nc.sync.dma_start(asm2[:, 0:DOUT], ag2[0, :, :])
                nc.sync.dma_start(asm2[:, DOUT:2 * DOUT], ag2[1, :, :])
                dr = misc_p.tile([2, D3], FP, tag="drug")
                nc.vector.tensor_copy(dr[:, 0:2 * DOUT], asm2[:])
                nc.vector.tensor_copy(dr[:, 2 * DOUT:D3], pb2[0:2, :])
                sq = norm_p.tile([2, D3], FP, tag="sq2")
                ssq = norm_p.tile([2, 1], FP, tag="ssq2")
                nc.vector.tensor_tensor(sq[:], dr[:], dr[:], ALU.mult)
                nc.vector.tensor_reduce(ssq[:], sq[:], AxisListType.X,
                                        ALU.add)
                nrm = norm_p.tile([2, 1], FP, tag="nrm2")
                nc.scalar.activation(nrm[:], ssq[:], AF.Sqrt)
                nc.vector.tensor_scalar_max(nrm[:], nrm[:], EPS)
                rn = norm_p.tile([2, 1], FP, tag="rn2")
                nc.vector.reciprocal(rn[:], nrm[:])
                nc.vector.tensor_scalar(dr[:], dr[:], rn[:], None, ALU.mult)
                nc.scalar.mul(sq[:], dr[:], LEAK)
                nc.vector.tensor_max(dr[:], dr[:], sq[:])

                # ---- head: ypred = (a P1 P2) . (b P1) ----
                ND3 = D3 // P
                dT = misc_p.tile([P, ND3 * 2], FP, tag="dT")
                for d in range(ND3):
                    pt = psum_p.tile([P, 2], FP, tag="ps", name="phd")
                    nc.tensor.transpose(pt[:], dr[:, d * P:(d + 1) * P],
                                        ident[0:2, 0:2])
                    nc.vector.tensor_copy(dT[:, d * 2:(d + 1) * 2], pt[:])
                pw = psum_p.tile([P, 2], FP, tag="ps", name="pw")
                for d in range(ND3):
                    nc.tensor.matmul(pw[:], p1_t[:, d * DEC:(d + 1) * DEC],
                                     dT[:, d * 2:(d + 1) * 2],
                                     start=(d == 0), stop=(d == ND3 - 1))
                w_hd = misc_p.tile([P, 2], FP, tag="w_hd")
                nc.vector.tensor_copy(w_hd[:], pw[:])
                ptt = psum_p.tile([P, 1], FP, tag="ps", name="ptt")
                nc.tensor.matmul(ptt[:], p2_t[:], w_hd[:, 0:1], start=True,
                                 stop=True)
                t_sb = misc_p.tile([P, 1], FP, tag="t_sb")
                nc.vector.tensor_copy(t_sb[:], ptt[:])
                py_ = psum_p.tile([1, 1], FP, tag="ps", name="pyf")
                nc.tensor.matmul(py_[:], t_sb[:], w_hd[:, 1:2], start=True,
                                 stop=True)
                y_sb = misc_p.tile([1, 1], FP, tag="y_sb")
                nc.vector.tensor_copy(y_sb[:], py_[:])
                nc.sync.dma_start(y_d.ap(), y_sb[:])
            elif stop_phase > 5:
                _dump_and_done(hcur[0][0:1, 0:1])

    nc.compile()
    return nc


# ---------------------------------------------------------------------------
# Host-side input prep
# ---------------------------------------------------------------------------

def _pack_quarters(a16, w16, N):
    """Interleave 512-col quarters: [a[:,q] | w[:,q]] -> [N, 2N] fp16."""
    NQ = max(N // JB, 1)
    q = min(JB, N)
    out = np.empty((N, 2 * N), dtype=np.float16)
    for i in range(NQ):
        out[:, i * 2 * q:i * 2 * q + q] = a16[:, i * q:(i + 1) * q]
        out[:, i * 2 * q + q:(i + 1) * 2 * q] = w16[:, i * q:(i + 1) * q]
    return out


def make_in_maps(inputs: dict, n_cores: int, N: int = None):
    """Per-core input dicts. Core 2b = up path of batch b, 2b+1 = down."""
    if N is None:
        N = np.asarray(inputs["adj"]).shape[-1]
    f32c = lambda a: np.ascontiguousarray(np.asarray(a, dtype=np.float32))
    f16c = lambda a: np.ascontiguousarray(np.asarray(a).astype(np.float16))

    def bake_mask(w):
        w = np.array(w, dtype=np.float32)
        w[-2:, :] = 1.0
        w[:, -2:] = 1.0
        return w

    maps = []
    for c in range(n_cores):
        b, down = divmod(c, 2)
        if not down:
            A = np.asarray(inputs["adj"][b])
            IV = np.asarray(inputs["up_inv_deg"][b])
            was = [bake_mask(inputs[f"l{l}_up_adj_w"]) for l in range(3)]
            wcs = [inputs[f"l{l}_up_w"] for l in range(3)]
        else:
            A = np.asarray(inputs["adj"][b]).T
            IV = np.asarray(inputs["down_inv_deg"][b])
            was = [bake_mask(inputs[f"l{l}_down_adj_w"]).T for l in range(3)]
            wcs = [inputs[f"l{l}_down_w"] for l in range(3)]
        A16 = f16c(A)
        m = {
            "x": f16c(inputs["x"][b]),
            "adj": A16,
            "invT": f16c(IV.T),
            "ivrT": f16c(IV[-2:, :].T),
            "p1": f32c(inputs["parameter1"]),
            "p2": f32c(inputs["parameter2"]),
        }
        for l in range(2):
            m[f"wa{l}"] = f16c(was[l])
        # l2: prodT = adjT .* waT (transpose of this core's own matrices)
        m["aw2"] = _pack_quarters(
            np.ascontiguousarray(A16.T), f16c(was[2].T), N)
        for l in range(3):
            m[f"w{l}c"] = f32c(wcs[l])
            m[f"w{l}b"] = f32c(inputs[f"l{l}_bias"])
        maps.append(m)
    return maps


_nc_cache = {}


def _get_program(n_cores, N):
    key = (n_cores, N)
    if key not in _nc_cache:
        _nc_cache[key] = build_program(n_cores, N)
    return _nc_cache[key]


def kernel(**inputs) -> np.ndarray:
    n_cores = 8
    nc = _get_program(n_cores, N_FULL)
    in_maps = make_in_maps(inputs, n_cores)
    res = run_bass_kernel_spmd(nc, in_maps, core_ids=list(range(n_cores)))
    out = np.zeros((B, 1), dtype=np.float32)
    for b in range(B):
        out[b, 0] = res.results[2 * b]["ypred"][0, 0]
    return out


# revision 45
# speedup vs baseline: 2.3464x; 1.0035x over previous
"""BiGraphSAGEDecoder Trainium2 kernel (v2 — restructured).

Sharding: 8 cores = 4 batches x {up-path, down-path}. One SPMD bass program;
up/down asymmetry handled by data (down cores get host-transposed matrices).

Key restructurings vs v1:
  - mm1 emits s^T directly (lhsT = h blocks, rhs = prod strips), enabling
    the reassociation x' = inv @ (s @ Wc): the big [N,N] matmul contracts
    against 256 cols instead of din=768.
  - layer 2 never computes s: x2 = ivr @ prod^T @ h is folded as
    u = ivr @ prodT (prodT streamed from host-transposed adj/wadj),
    w = u^T, x2 = w^T @ h.
  - full bias chunk computed locally (no bias in the exchange); AllGather
    moves only the 256-col own chunk per rank, in fp16.
  - dtypes: adj/wadj/inv fp16 in DRAM, prod fp16, h fp16, y fp16,
    sT fp32r, weights fp32r, psum fp32. Validated max rel err 1.9e-3
    vs fp32 reference (tolerance 2e-2).

Math per layer (per core, its path):
  prod = adj .* wadj_baked                  (DVE, fp16)
  sT   = (prod^T @ h)^T  via lhsT=h         (PE)
  y    = s @ Wc                             (PE, lhsT = sT blocks)
  own  = inv @ y                            (PE, lhsT = invT blocks)
  bias = h @ Wb (full)                      (PE, via PE-transposed h)
  exchange own chunks (2-rank AllGather, two halves, fp16)
  h' = lrelu(cat / max(||cat||, eps))       (DVE/scalar/gpsimd)
"""

import os
import sys
import types
import contextlib

sys.path.insert(0, "/opt/trn_rl_repo")

import numpy as np

import concourse.bass as bass
import concourse.tile as tile
from concourse import mybir, bacc
from concourse.mybir import AxisListType
from concourse.masks import make_identity
from concourse.bass_utils import run_bass_kernel_spmd

FP = mybir.dt.float32
FPR = mybir.dt.float32r
F16 = mybir.dt.float16
F8 = mybir.dt.float8e4
AF = mybir.ActivationFunctionType
ALU = mybir.AluOpType

# ---------------------------------------------------------------------------
# Environment patches (required for this container's toolchain)
# ---------------------------------------------------------------------------


def install_ntff_shim():
    """antenv.axon_hooks is absent in this image; provide it so trace=True
    profiling works (used by test.py, harmless otherwise)."""
    try:
        import antenv.axon_hooks  # noqa: F401
        return
    except ImportError:
        pass
    try:
        import antenv
    except ImportError:
        return
    mod = types.ModuleType("antenv.axon_hooks")
    _holder = {"hook": None}
    mod.set_axon_ntff_profile_hook = lambda h: _holder.__setitem__("hook", h)
    mod.get_axon_ntff_profile_hook = lambda: _holder["hook"]
    sys.modules["antenv.axon_hooks"] = mod
    antenv.axon_hooks = mod
    try:
        from trn_agent_boot.trn_boot import _ntff_profile_via_ctypes

        hook = _ntff_profile_via_ctypes("/opt/axon/libaxon_pjrt.so")
        if hook is not None:
            mod.set_axon_ntff_profile_hook(hook)
    except Exception:
        pass


install_ntff_shim()

if os.environ.get("KGSD_LDW_OPT", "0") != "0":
    # let walrus dedup back-to-back LDWEIGHTS
    import concourse.bass_utils as _bu
    _orig_run_command = _bu.run_command

    def _patched_run_command(argv, **kw):
        argv = ["--enable-ldw-opt=true" if a == "--enable-ldw-opt=false"
                else a for a in argv]
        return _orig_run_command(argv, **kw)

    _bu.run_command = _patched_run_command

# ---------------------------------------------------------------------------
# Problem constants
# ---------------------------------------------------------------------------

N_FULL = 2048
B = 4
P = 128
DOUT = 256     # per-path cat chunk width (also bias width)
DEC = 128
DINS = (256, 768, 768)   # per-layer input dims
D3 = 768
EPS = 1e-12
LEAK = 0.1
JB = 512       # mm1 / x' column block (one PSUM bank of fp32)


# ---------------------------------------------------------------------------
# Program builder
# ---------------------------------------------------------------------------


def build_program(n_cores: int, N: int = N_FULL, stop_phase: int = 99):
    """Build the SPMD bass program.

    stop_phase (debug): 1=x load, 2=l0 mm1, 3=l0 y, 4=l0 x', 5=l0 done,
    6=l1 done, 7=full.
    """
    NT = N // P              # 128-row tiles
    NJB = N // JB            # mm1/x' column blocks
    HALF = NT // 2
    NQ = max(N // JB, 1)     # aw quarter count (512-wide pairs)

    nc = bacc.Bacc("TRN2", target_bir_lowering=False, debug=False,
                   num_devices=n_cores)

    # --- DRAM I/O ---
    x_d = nc.dram_tensor("x", [N, DINS[0]], F16, kind="ExternalInput")
    adj_d = nc.dram_tensor("adj", [N, N], F16, kind="ExternalInput")
    wa_d = [nc.dram_tensor(f"wa{l}", [N, N], F8, kind="ExternalInput")
            for l in range(2)]
    adjT_d = nc.dram_tensor("adjT", [N, N], F16, kind="ExternalInput")
    waT_d = nc.dram_tensor("waT", [N, N], F8, kind="ExternalInput")
    invT_d = nc.dram_tensor("invT", [N, N], F16, kind="ExternalInput")
    ivrT_d = nc.dram_tensor("ivrT", [N, 2], F16, kind="ExternalInput")
    wc_d = [nc.dram_tensor(f"w{l}c", [DINS[l], DOUT], FP, kind="ExternalInput")
            for l in range(3)]
    wb_d = [nc.dram_tensor(f"w{l}b", [DINS[l], DOUT], FP, kind="ExternalInput")
            for l in range(3)]
    p1_d = nc.dram_tensor("p1", [D3, DEC], FP, kind="ExternalInput")
    p2_d = nc.dram_tensor("p2", [DEC, DEC], FP, kind="ExternalInput")
    y_d = nc.dram_tensor("ypred", [1, 1], FP, kind="ExternalOutput")
    dbg_d = (nc.dram_tensor("dbg", [N, D3], FP, kind="ExternalOutput")
             if stop_phase < 99 else None)

    groups = [[i, i + 1] for i in range(0, n_cores, 2)]

    with tile.TileContext(nc) as tc:
        with contextlib.ExitStack() as ctx:
            const_p = ctx.enter_context(tc.tile_pool(name="const", bufs=1))
            h_p = ctx.enter_context(tc.tile_pool(name="h", bufs=1))
            sT_p = ctx.enter_context(tc.tile_pool(name="sT", bufs=1))
            y_p = ctx.enter_context(tc.tile_pool(name="y", bufs=1))
            aw_p = ctx.enter_context(tc.tile_pool(name="aw", bufs=4))
            prod_p = ctx.enter_context(tc.tile_pool(name="prod", bufs=4))
            inv_p = ctx.enter_context(tc.tile_pool(name="invs", bufs=2))
            hT_p = ctx.enter_context(tc.tile_pool(name="hT", bufs=2))
            stg_p = ctx.enter_context(tc.tile_pool(name="stg", bufs=2))
            norm_p = ctx.enter_context(tc.tile_pool(name="norm", bufs=2))
            misc_p = ctx.enter_context(tc.tile_pool(name="misc", bufs=1))
            psum_p = ctx.enter_context(
                tc.tile_pool(name="psum", bufs=8, space="PSUM"))
            dram_p = ctx.enter_context(
                tc.tile_pool(name="dram", bufs=2, space="DRAM"))

            ident = const_p.tile([P, P], FP, tag="ident")
            make_identity(nc, ident)
            ident16 = const_p.tile([P, P], F16, tag="ident16")
            nc.scalar.copy(ident16[:], ident[:])

            # --- constant tiles (loads deferred; see _load_w) ---
            wc_t, wb_t = [], []
            for l in range(3):
                ND = DINS[l] // P
                wct = const_p.tile([P, ND * DOUT], FPR, tag=f"wc{l}")
                wbt = const_p.tile([P, ND * DOUT], FPR, tag=f"wb{l}")
                wc_t.append(wct)
                wb_t.append(wbt)

            def _load_w(l):
                ND = DINS[l] // P
                for d in range(ND):
                    nc.scalar.dma_start(
                        wc_t[l][:, d * DOUT:(d + 1) * DOUT],
                        wc_d[l].ap()[d * P:(d + 1) * P, :].bitcast(FPR))
                    nc.scalar.dma_start(
                        wb_t[l][:, d * DOUT:(d + 1) * DOUT],
                        wb_d[l].ap()[d * P:(d + 1) * P, :].bitcast(FPR))

            _load_w(0)
            p1_t = const_p.tile([P, (D3 // P) * DEC], FP, tag="p1")
            p2_t = const_p.tile([P, DEC], FP, tag="p2")
            ivrT_t = const_p.tile([P, NT, 2], F16, tag="ivrT")
            nc.scalar.dma_start(
                ivrT_t[:],
                ivrT_d.ap().rearrange("(a p) c -> p a c", p=P))

            # --- h double generation (fp16) ---
            h_e = [h_p.tile([P, D3], F16, tag=f"he{k}", name="h_e")
                   for k in range(NT)]
            h_o = [h_p.tile([P, D3], F16, tag=f"ho{k}", name="h_o")
                   for k in range(NT)]
            # x loads are interleaved with the first mm1 strip loads below

            sT = [sT_p.tile([P, N], FPR, tag=f"sT{d}", name="sT")
                  for d in range(6)]
            y_t = [y_p.tile([P, DOUT], F16, tag=f"y{j}", name="y_t")
                   for j in range(NT)]
            w_sb = misc_p.tile([P, 2 * NT], F16, tag="w_sb")

            def _dump_and_done(src_ap):
                y_sb0 = misc_p.tile([1, 1], FP, tag="y_dbg")
                nc.vector.tensor_copy(y_sb0[:], src_ap)
                nc.sync.dma_start(y_d.ap(), y_sb0[:])

            # l2 prefold bursts: split quarters between the two boundaries;
            # steps are interleaved into the bias loops (one step per it)
            burst_q = [list(range(NQ))[:(NQ + 1) // 2],
                       list(range(NQ))[(NQ + 1) // 2:]]
            pu_live = {}

            def burst_step(qg, jj):
                if jj == 0:
                    pu_live[qg] = psum_p.tile([P, JB], FP, tag="ps",
                                              name="pu")
                pu = pu_live[qg]
                rsl = slice(jj * 2 * P, (jj + 1) * 2 * P)
                csl = slice(qg * JB, (qg + 1) * JB)
                a2t = aw_p.tile([P, 2, JB], F16, tag="adj")
                nc.scalar.dma_start(
                    a2t[:], adjT_d.ap()[rsl, csl]
                    .rearrange("(a p) c -> p a c", p=P))
                w8t = aw_p.tile([P, 2, JB], F8, tag="wa8", bufs=2)
                nc.scalar.dma_start(
                    w8t[:], waT_d.ap()[rsl, csl]
                    .rearrange("(a p) c -> p a c", p=P))
                w2t = aw_p.tile([P, 2, JB], F16, tag="wa")
                nc.vector.tensor_copy(w2t[:], w8t[:])
                pr = prod_p.tile([P, 2, JB], F16, tag="prod")
                nc.vector.tensor_tensor(pr[:], a2t[:], w2t[:], ALU.mult)
                for a in range(2):
                    j = jj * 2 + a
                    nc.tensor.matmul(pu[0:2, :], ivrT_t[:, j, :],
                                     pr[:, a, :], start=(j == 0),
                                     stop=(j == NT - 1))
                if jj == NT // 2 - 1:
                    usb = misc_p.tile([2, JB], F16, tag="usb")
                    nc.scalar.copy(usb[:], pu[0:2, :])
                    for kk in range(JB // P):
                        k = qg * (JB // P) + kk
                        if k >= NT:
                            break
                        ptw = psum_p.tile([P, 2], F16, tag="ps", name="ptw")
                        nc.tensor.transpose(
                            ptw[:], usb[:, kk * P:(kk + 1) * P],
                            ident16[0:2, 0:2])
                        nc.vector.tensor_copy(w_sb[:, 2 * k:2 * k + 2],
                                              ptw[:])

            if stop_phase <= 1:
                _dump_and_done(h_e[0][0:1, 0:1])

            hcur, hnxt = h_e, h_o
            n_layers = 0 if stop_phase <= 1 else (2 if stop_phase <= 6 else 2)
            for l in range(2):
                if stop_phase <= 1:
                    break
                din = DINS[l]
                ND = din // P

                # ---- mm1: sT = (prod^T @ h)^T ----
                for jb in range(NJB):
                    pss = [psum_p.tile([P, JB], FP, tag="ps", name="pmm1")
                           for _ in range(ND)]
                    for kk in range(NT // 2):
                        if l == 0 and jb == 0:
                            for a in range(2):
                                kt = kk * 2 + a
                                nc.sync.dma_start(
                                    h_e[kt][:, 0:DINS[0]],
                                    x_d.ap()[kt * P:(kt + 1) * P, :])
                        rsl = slice(kk * 2 * P, (kk + 1) * 2 * P)
                        csl = slice(jb * JB, (jb + 1) * JB)
                        adt = aw_p.tile([P, 2, JB], F16, tag="adj")
                        nc.sync.dma_start(
                            adt[:], adj_d.ap()[rsl, csl]
                            .rearrange("(a p) c -> p a c", p=P))
                        wa8 = aw_p.tile([P, 2, JB], F8, tag="wa8", bufs=2)
                        nc.sync.dma_start(
                            wa8[:], wa_d[l].ap()[rsl, csl]
                            .rearrange("(a p) c -> p a c", p=P))
                        wat = aw_p.tile([P, 2, JB], F16, tag="wa")
                        nc.vector.tensor_copy(wat[:], wa8[:])
                        pr = prod_p.tile([P, 2, JB], F16, tag="prod")
                        nc.vector.tensor_tensor(pr[:], adt[:], wat[:],
                                                ALU.mult)
                        for a in range(2):
                            k = kk * 2 + a
                            for d in range(ND):
                                nc.tensor.matmul(
                                    pss[d][:], hcur[k][:, d * P:(d + 1) * P],
                                    pr[:, a, :], start=(k == 0),
                                    stop=(k == NT - 1))
                    for d in range(ND):
                        dst = sT[d][:, jb * JB:(jb + 1) * JB]
                        if d % 2 == 0:
                            nc.scalar.copy(dst, pss[d][:])
                        else:
                            nc.vector.tensor_copy(dst, pss[d][:])

                if stop_phase == 2 and l == 0:
                    _dump_and_done(sT[0][0:1, 0:1])
                    break

                # ---- y = s @ Wc  (fp16) ----
                for j in range(NT):
                    py = psum_p.tile([P, DOUT], FP, tag="ps", name="py")
                    for d in range(ND):
                        nc.tensor.matmul(
                            py[:], sT[d][:, j * P:(j + 1) * P],
                            wc_t[l][:, d * DOUT:(d + 1) * DOUT],
                            start=(d == 0), stop=(d == ND - 1))
                    nc.scalar.copy(y_t[j][:], py[:])

                if stop_phase == 3 and l == 0:
                    _dump_and_done(y_t[0][0:1, 0:1])
                    break

                # ---- x' = inv @ y : own cat chunk ----
                pxs = [psum_p.tile([P, JB], FP, tag="ps", name="px")
                       for _ in range((NT + 1) // 2)]
                for px in pxs:
                    nc.vector.memset(px[:], 0.0)
                for j in range(NT):
                    ivt = inv_p.tile([P, N], F16, tag="inv", bufs=3)
                    nc.sync.dma_start(ivt[:],
                                      invT_d.ap()[j * P:(j + 1) * P, :])
                    for i in range(NT):
                        px = pxs[i // 2]
                        sl = slice((i % 2) * DOUT, (i % 2 + 1) * DOUT)
                        nc.tensor.matmul(
                            px[:, sl], ivt[:, i * P:(i + 1) * P], y_t[j][:],
                            start=False, stop=(j == NT - 1),
                            skip_group_check=True)

                if stop_phase == 4 and l == 0:
                    _dump_and_done(y_t[0][0:1, 0:1])
                    break

                # ---- stage + AllGather (two halves) ----
                stage_d = [dram_p.tile([N // 2, DOUT], F16, tag=f"stg{hh}",
                                       name="stage_d") for hh in range(2)]
                GW = min(4, HALF)  # i-blocks per staged DMA
                for g in range(NT // GW):
                    st = stg_p.tile([P, GW, DOUT], F16, tag="stg")
                    for a2 in range(GW // 2):
                        nc.scalar.copy(
                            st[:, 2 * a2:2 * a2 + 2, :],
                            pxs[(g * GW) // 2 + a2][:])
                    hh, go = divmod(g, HALF // GW)
                    nc.scalar.dma_start(
                        stage_d[hh][go * GW * P:(go + 1) * GW * P, :]
                        .rearrange("(a p) c -> p a c", p=P),
                        st[:])
                ag_t = []
                for hh in range(2):
                    agt = dram_p.tile([2, N // 2, DOUT], F16, tag=f"ag{hh}",
                                      name="ag_t")
                    nc.gpsimd.collective_compute(
                        "AllGather", ALU.bypass, replica_groups=groups,
                        ins=[stage_d[hh].opt()], outs=[agt.opt()])
                    ag_t.append(agt)

                # ---- weight prefetch for later phases (scalar queue) ----
                if l == 0:
                    _load_w(1)
                else:
                    _load_w(2)
                    for d in range(D3 // P):
                        nc.scalar.dma_start(p1_t[:, d * DEC:(d + 1) * DEC],
                                            p1_d.ap()[d * P:(d + 1) * P, :])
                    nc.scalar.dma_start(p2_t[:], p2_d.ap())

                # ---- bias = h @ Wb (full, local) -> h' cols 512:768 ----
                # (one l2-prefold burst step interleaved per it)
                bsteps = [(qg, jj) for qg in burst_q[l]
                          for jj in range(NT // 2)]
                for it in range(NT):
                    pb = psum_p.tile([P, DOUT], FP, tag="ps", name="pb")
                    for dd in range(ND // 2):
                        ptr = psum_p.tile([P, 2 * P], F16, tag="ps",
                                          name="ptr")
                        nc.tensor.matmul(
                            ptr[:, 0:P], hcur[it][:, 2 * dd * P:
                                                  (2 * dd + 1) * P],
                            ident16[:], is_transpose=True,
                            start=True, stop=False, skip_group_check=True)
                        nc.tensor.matmul(
                            ptr[:, P:2 * P],
                            hcur[it][:, (2 * dd + 1) * P:(2 * dd + 2) * P],
                            ident16[:], is_transpose=True,
                            start=False, stop=True, skip_group_check=True)
                        hTt = hT_p.tile([P, 2 * P], FPR, tag="hT")
                        nc.vector.tensor_copy(hTt[:], ptr[:])
                        for a in range(2):
                            d = 2 * dd + a
                            nc.tensor.matmul(
                                pb[:], hTt[:, a * P:(a + 1) * P],
                                wb_t[l][:, d * DOUT:(d + 1) * DOUT],
                                start=(d == 0), stop=(d == ND - 1))
                    nc.scalar.copy(hnxt[it][:, 2 * DOUT:3 * DOUT], pb[:])
                    if it < len(bsteps):
                        burst_step(*bsteps[it])

                # ---- leftover burst steps (if NT < steps) ----
                for st_ in bsteps[NT:]:
                    burst_step(*st_)

                # ---- assemble + normalize + lrelu -> h' ----
                # (for l1: x2 = w^T @ h matmuls trail each tile's norm)
                if l == 1 and stop_phase > 6:
                    psA = psum_p.tile([P, JB], FP, tag="ps", name="psA")
                    psB = psum_p.tile([P, JB], FP, tag="ps", name="psB")
                for it in range(NT):
                    hh, io = divmod(it, HALF)
                    ag = ag_t[hh]
                    ht = hnxt[it]
                    nc.gpsimd.dma_start(
                        ht[:, 0:2 * DOUT],
                        ag[:, io * P:(io + 1) * P, :]
                        .rearrange("r p c -> p r c"))
                    if stop_phase == 4.75 and l == 0:
                        dbf = misc_p.tile([P, D3], FP, tag="dbf", bufs=2)
                        nc.vector.tensor_copy(dbf[:], ht[:])
                        nc.sync.dma_start(
                            dbg_d.ap()[it * P:(it + 1) * P, :], dbf[:])
                        continue
                    sqs = norm_p.tile([P, D3], F16, tag="sq")
                    ssq = norm_p.tile([P, 1], FP, tag="ssq")
                    if it % 2 == 0:
                        nc.scalar.activation(sqs[:], ht[:], AF.Square,
                                             accum_out=ssq[:])
                    else:
                        nc.vector.scalar_tensor_tensor(
                            sqs[:], ht[:], 1.0, ht[:], ALU.mult, ALU.mult,
                            accum_out=ssq[:])
                    nrm = norm_p.tile([P, 1], FP, tag="nrm")
                    nc.scalar.activation(nrm[:], ssq[:], AF.Sqrt)
                    nc.vector.tensor_scalar_max(nrm[:], nrm[:], EPS)
                    rn = norm_p.tile([P, 1], FP, tag="rn")
                    nc.vector.reciprocal(rn[:], nrm[:])
                    nc.scalar.mul(ht[:], ht[:], rn[:])
                    nc.vector.scalar_tensor_tensor(
                        ht[:], ht[:], LEAK, ht[:], ALU.mult, ALU.max)
                    if l == 1 and stop_phase > 6:
                        nc.tensor.matmul(psA[0:2, :],
                                         w_sb[:, 2 * it:2 * it + 2],
                                         ht[:, 0:JB], start=(it == 0),
                                         stop=(it == NT - 1))
                        nc.tensor.matmul(psB[0:2, 0:D3 - JB],
                                         w_sb[:, 2 * it:2 * it + 2],
                                         ht[:, JB:D3], start=(it == 0),
                                         stop=(it == NT - 1))

                if stop_phase == 4.75 and l == 0:
                    break
                hcur, hnxt = hnxt, hcur
                if stop_phase == 5 and l == 0:
                    _dump_and_done(hcur[0][0:1, 0:1])
                    for it in range(NT):
                        dbf = misc_p.tile([P, D3], FP, tag="dbf", bufs=2)
                        nc.vector.tensor_copy(dbf[:], hcur[it][:])
                        nc.sync.dma_start(
                            dbg_d.ap()[it * P:(it + 1) * P, :], dbf[:])
                    break

            do_tail = stop_phase > 6
            if do_tail:
                # ---- l2 tail (x2 psums already accumulated in norm loop) ----
                x2sb = misc_p.tile([2, D3], F16, tag="x2sb")
                nc.scalar.copy(x2sb[:, 0:JB], psA[0:2, :])
                nc.scalar.copy(x2sb[:, JB:D3], psB[0:2, 0:D3 - JB])

                # own2 = x2 @ Wc2
                pc2 = psum_p.tile([P, DOUT], FP, tag="ps", name="pc2")
                for d in range(D3 // P):
                    ptx = psum_p.tile([P, 2], F16, tag="ps", name="ptx")
                    nc.tensor.transpose(
                        ptx[:], x2sb[:, d * P:(d + 1) * P],
                        ident16[0:2, 0:2])
                    x2T = hT_p.tile([P, 2], FPR, tag="x2T")
                    nc.vector.tensor_copy(x2T[:], ptx[:])
                    nc.tensor.matmul(pc2[0:2, :], x2T[:],
                                     wc_t[2][:, d * DOUT:(d + 1) * DOUT],
                                     start=(d == 0), stop=(d == D3 // P - 1))

                # stage2 + AG2 launched before bias2 (AG flies during bias2)
                stg2 = stg_p.tile([2, DOUT], F16, tag="stg")
                nc.scalar.copy(stg2[:], pc2[0:2, :])
                st2d = dram_p.tile([2, DOUT], F16, tag="stg2")
                nc.scalar.dma_start(st2d[:], stg2[:])
                ag2 = dram_p.tile([2, 2, DOUT], F16, tag="ag2")
                nc.gpsimd.collective_compute(
                    "AllGather", ALU.bypass, replica_groups=groups,
                    ins=[st2d.opt()], outs=[ag2.opt()])

                # bias2 = hdrug @ Wb2 (hdrug via DRAM bounce, post-norm rows)
                bounce = dram_p.tile([2, D3], F16, tag="bounce")
                nc.sync.dma_start(bounce[:], hcur[NT - 1][P - 2:P, :])
                hdr = misc_p.tile([2, D3], F16, tag="hdrug")
                nc.sync.dma_start(hdr[:], bounce[:])
                pb2 = psum_p.tile([P, DOUT], FP, tag="ps", name="pb2")
                for d in range(D3 // P):
                    ptr = psum_p.tile([P, 2], F16, tag="ps", name="ptr2")
                    nc.tensor.transpose(ptr[:], hdr[:, d * P:(d + 1) * P],
                                        ident16[0:2, 0:2])
                    hTt = hT_p.tile([P, 2], FPR, tag="x2T")
                    nc.vector.tensor_copy(hTt[:], ptr[:])
                    nc.tensor.matmul(pb2[0:2, :], hTt[:],
                                     wb_t[2][:, d * DOUT:(d + 1) * DOUT],
                                     start=(d == 0), stop=(d == D3 // P - 1))

                # assemble + norm
                asm2 = stg_p.tile([2, 2 * DOUT], F16, tag="asm2")
                nc.sync.dma_start(asm2[:, 0:DOUT], ag2[0, :, :])
                nc.sync.dma_start(asm2[:, DOUT:2 * DOUT], ag2[1, :, :])
                dr = misc_p.tile([2, D3], FP, tag="drug")
                nc.vector.tensor_copy(dr[:, 0:2 * DOUT], asm2[:])
                nc.vector.tensor_copy(dr[:, 2 * DOUT:D3], pb2[0:2, :])
                sq = norm_p.tile([2, D3], FP, tag="sq2")
                ssq = norm_p.tile([2, 1], FP, tag="ssq2")
                nc.vector.tensor_tensor(sq[:], dr[:], dr[:], ALU.mult)
                nc.vector.tensor_reduce(ssq[:], sq[:], AxisListType.X,
                                        ALU.add)
                nrm = norm_p.tile([2, 1], FP, tag="nrm2")
                nc.scalar.activation(nrm[:], ssq[:], AF.Sqrt)
                nc.vector.tensor_scalar_max(nrm[:], nrm[:], EPS)
                rn = norm_p.tile([2, 1], FP, tag="rn2")
                nc.vector.reciprocal(rn[:], nrm[:])
                nc.vector.tensor_scalar(dr[:], dr[:], rn[:], None, ALU.mult)
                nc.scalar.mul(sq[:], dr[:], LEAK)
                nc.vector.tensor_max(dr[:], dr[:], sq[:])

                # ---- head: ypred = (a P1 P2) . (b P1) ----
                ND3 = D3 // P
                dT = misc_p.tile([P, ND3 * 2], FP, tag="dT")
                for d in range(ND3):
                    pt = psum_p.tile([P, 2], FP, tag="ps", name="phd")
                    nc.tensor.transpose(pt[:], dr[:, d * P:(d + 1) * P],
                                        ident[0:2, 0:2])
                    nc.vector.tensor_copy(dT[:, d * 2:(d + 1) * 2], pt[:])
                pw = psum_p.tile([P, 2], FP, tag="ps", name="pw")
                for d in range(ND3):
                    nc.tensor.matmul(pw[:], p1_t[:, d * DEC:(d + 1) * DEC],
                                     dT[:, d * 2:(d + 1) * 2],
                                     start=(d == 0), stop=(d == ND3 - 1))
                w_hd = misc_p.tile([P, 2], FP, tag="w_hd")
                nc.vector.tensor_copy(w_hd[:], pw[:])
                ptt = psum_p.tile([P, 1], FP, tag="ps", name="ptt")
                nc.tensor.matmul(ptt[:], p2_t[:], w_hd[:, 0:1], start=True,
                                 stop=True)
                t_sb = misc_p.tile([P, 1], FP, tag="t_sb")
                nc.vector.tensor_copy(t_sb[:], ptt[:])
                py_ = psum_p.tile([1, 1], FP, tag="ps", name="pyf")
                nc.tensor.matmul(py_[:], t_sb[:], w_hd[:, 1:2], start=True,
                                 stop=True)
                y_sb = misc_p.tile([1, 1], FP, tag="y_sb")
                nc.vector.tensor_copy(y_sb[:], py_[:])
                nc.sync.dma_start(y_d.ap(), y_sb[:])
            elif stop_phase > 5:
                _dump_and_done(hcur[0][0:1, 0:1])

    nc.compile()
    return nc


# ---------------------------------------------------------------------------
# Host-side input prep
# ---------------------------------------------------------------------------

def _pack_quarters(a16, w16, N):
    """Interleave 512-col quarters: [a[:,q] | w[:,q]] -> [N, 2N] fp16."""
    NQ = max(N // JB, 1)
    q = min(JB, N)
    out = np.empty((N, 2 * N), dtype=np.float16)
    for i in range(NQ):
        out[:, i * 2 * q:i * 2 * q + q] = a16[:, i * q:(i + 1) * q]
        out[:, i * 2 * q + q:(i + 1) * 2 * q] = w16[:, i * q:(i + 1) * q]
    return out


def make_in_maps(inputs: dict, n_cores: int, N: int = None):
    """Per-core input dicts. Core 2b = up path of batch b, 2b+1 = down."""
    if N is None:
        N = np.asarray(inputs["adj"]).shape[-1]
    f32c = lambda a: np.ascontiguousarray(np.asarray(a, dtype=np.float32))
    f16c = lambda a: np.ascontiguousarray(np.asarray(a).astype(np.float16))

    def bake_mask(w):
        w = np.array(w, dtype=np.float32)
        w[-2:, :] = 1.0
        w[:, -2:] = 1.0
        return w

    maps = []
    for c in range(n_cores):
        b, down = divmod(c, 2)
        if not down:
            A = np.asarray(inputs["adj"][b])
            IV = np.asarray(inputs["up_inv_deg"][b])
            was = [bake_mask(inputs[f"l{l}_up_adj_w"]) for l in range(3)]
            wcs = [inputs[f"l{l}_up_w"] for l in range(3)]
        else:
            A = np.asarray(inputs["adj"][b]).T
            IV = np.asarray(inputs["down_inv_deg"][b])
            was = [bake_mask(inputs[f"l{l}_down_adj_w"]).T for l in range(3)]
            wcs = [inputs[f"l{l}_down_w"] for l in range(3)]
        import ml_dtypes
        f8c = lambda a: np.ascontiguousarray(
            np.asarray(a).astype(ml_dtypes.float8_e4m3))
        A16 = f16c(A)
        m = {
            "x": f16c(inputs["x"][b]),
            "adj": A16,
            "adjT": np.ascontiguousarray(A16.T),
            "invT": f16c(IV.T),
            "ivrT": f16c(IV[-2:, :].T),
            "p1": f32c(inputs["parameter1"]),
            "p2": f32c(inputs["parameter2"]),
        }
        for l in range(2):
            m[f"wa{l}"] = f8c(was[l])
        # l2: prodT = adjT .* waT (transpose of this core's own matrices)
        m["waT"] = f8c(was[2].T)
        for l in range(3):
            m[f"w{l}c"] = f32c(wcs[l])
            m[f"w{l}b"] = f32c(inputs[f"l{l}_bias"])
        maps.append(m)
    return maps


_nc_cache = {}


def _get_program(n_cores, N):
    key = (n_cores, N)
    if key not in _nc_cache:
        _nc_cache[key] = build_program(n_cores, N)
    return _nc_cache[key]


def kernel(**inputs) -> np.ndarray:
    n_cores = 8
    nc = _get_program(n_cores, N_FULL)
    in_maps = make_in_maps(inputs, n_cores)
    res = run_bass_kernel_spmd(nc, in_maps, core_ids=list(range(n_cores)))
    out = np.zeros((B, 1), dtype=np.float32)
    for b in range(B):
        out[b, 0] = res.results[2 * b]["ypred"][0, 0]
    return out
